# revision 5
# baseline (speedup 1.0000x reference)
"""MemoryNet kernel for 8 TRN2 NeuronCores (Bass/Tile).

Reference (single-device):
    key = softmax(mem @ fk_w.T + fk_b, axis=-1)      # [J, D]
    val = relu(mem @ fv_w.T + fv_b)                  # [J, D]
    att = softmax(k @ key.T, axis=-1)                # [N, J]
    out = att @ val                                  # [N, D]
with J=4096 (num_mem), MD=512 (mem_dim), D=1024 (inp_dim), N=32768.

Sharding: data-parallel over rows of k (N) across 8 cores; mem + weights
replicated on every core. Each core computes out rows for its shard; the
host concatenates.

Per-core algorithm (all big matmuls in bf16, fp32 accumulation):
  Phase 0 (replicated derivation, transposed layout):
    memT[m,j], fk_wT[m,d], fv_wT[m,d] via PE transposes.
    ekT[d,j]  = exp(fk_wT.T @ memT + fk_b[d])        (softmax numerator, no
                max subtraction -- |logits| <= ~4 for this problem family)
    csum[j]   = sum_d ekT  (ones-vector matmul over partitions)
    ekT      *= (1/csum)[j]  -> normalized key, transposed, bf16
    val[j,dd] = relu(memT.T @ fv_wT + fv_b)  (fv_b added via rank-1 matmul)
  Phase 1 (per chunk of NCHUNK k-rows):
    kT[d,n]   via PE transposes of DMA'd k rows
    scoresT[j-tile, n] = ekT.T @ kT   (accumulate over d in PSUM)
    E = exp(scoresT)  (bf16, in SBUF; scores are O(0.2) so exp is safe)
    denom[n]  = sum_j E  (ones matmul), r = 1/denom, transposed to [n,1]
    outp[n-tile, dd] = E.T @ val  (accumulate over j in PSUM), scaled by r
"""

import numpy as np

P = 128
J = 4096      # num_mem
MD = 512      # mem_dim
D = 1024      # inp_dim
NTOT = 32768  # total k rows
NCORES = 8
S = NTOT // NCORES   # k rows per core
NCHUNK = 256         # k rows processed per phase-1 chunk

_CACHE = {}


def _build():
    import concourse.bass as bass
    import concourse.tile as tile
    from concourse import bacc, mybir

    f32 = mybir.dt.float32
    bf16 = mybir.dt.bfloat16
    AF = mybir.ActivationFunctionType

    nc = bacc.Bacc("TRN2", target_bir_lowering=False, debug=False,
                   num_devices=NCORES)

    k_d = nc.dram_tensor("k", [S, D], f32, kind="ExternalInput").ap()
    mem_d = nc.dram_tensor("mem", [J, MD], f32, kind="ExternalInput").ap()
    fkw_d = nc.dram_tensor("fk_w", [D, MD], f32, kind="ExternalInput").ap()
    fkb_d = nc.dram_tensor("fk_b", [D], f32, kind="ExternalInput").ap()
    fvw_d = nc.dram_tensor("fv_w", [D, MD], f32, kind="ExternalInput").ap()
    fvb_d = nc.dram_tensor("fv_b", [D], f32, kind="ExternalInput").ap()
    id_d = nc.dram_tensor("ident", [P, P], f32, kind="ExternalInput").ap()
    out_d = nc.dram_tensor("out", [S, D], f32, kind="ExternalOutput").ap()

    JT = J // P        # 32 j-tiles
    DT = D // P        # 8 d-tiles
    MT = MD // P       # 4 m-tiles
    NC_ = S // NCHUNK  # phase-1 chunks
    NS = NCHUNK // P   # n-subtiles per chunk

    with tile.TileContext(nc) as tc:
        from contextlib import ExitStack
        ctx = ExitStack()
        with ctx:
            persist = ctx.enter_context(tc.tile_pool(name="persist", bufs=1))
            ps_s = ctx.enter_context(tc.tile_pool(name="ps_s", bufs=3, space="PSUM"))
            ps_o = ctx.enter_context(tc.tile_pool(name="ps_o", bufs=2, space="PSUM"))
            ps_t = ctx.enter_context(tc.tile_pool(name="ps_t", bufs=2, space="PSUM"))

            # persistent tiles
            ekT = [persist.tile([P, J], bf16, tag=f"ekT{d}", name=f"ekT{d}")
                   for d in range(DT)]
            val = persist.tile([P, JT, D], bf16, tag="val")
            ident = persist.tile([P, P], f32, tag="ident")
            ones_c16 = persist.tile([P, 1], bf16, tag="ones_c")   # colsum lhsT
            ones_r16 = persist.tile([1, P], bf16, tag="ones_r")   # rank-1 bias lhsT
            ones_r32 = persist.tile([1, P], f32, tag="ones_r32")  # c-broadcast lhsT
            fkbT = persist.tile([P, DT], f32, tag="fkbT")

            nc.sync.dma_start(out=ident, in_=id_d)
            nc.vector.memset(ones_c16, 1.0)
            nc.vector.memset(ones_r16, 1.0)
            nc.vector.memset(ones_r32, 1.0)
            # fk_b -> per-partition layout: fkbT[p, t] = fk_b[t*128 + p]
            nc.sync.dma_start(out=fkbT, in_=fkb_d.rearrange("(t p) -> p t", p=P))

            # ---------------- Phase 0: key/val derivation ----------------
            with tc.tile_pool(name="p0", bufs=1) as p0, \
                 tc.tile_pool(name="p0st", bufs=2) as p0st:
                memT = [p0.tile([P, J], bf16, tag=f"memT{m}", name=f"memT{m}")
                        for m in range(MT)]
                fkT = [p0.tile([P, D], bf16, tag=f"fkT{m}", name=f"fkT{m}")
                       for m in range(MT)]
                fvT = [p0.tile([P, D], bf16, tag=f"fvT{m}", name=f"fvT{m}")
                       for m in range(MT)]
                fvb16 = p0.tile([1, D], bf16, tag="fvb16")

                fvb32 = p0st.tile([1, D], f32, tag="fvb32")
                nc.sync.dma_start(out=fvb32, in_=fvb_d.rearrange("(a d) -> a d", a=1))
                nc.vector.tensor_copy(out=fvb16, in_=fvb32)

                # transpose mem -> memT (bf16)
                for jt in range(JT):
                    st = p0st.tile([P, MD], f32, tag="st")
                    nc.sync.dma_start(out=st, in_=mem_d[jt * P:(jt + 1) * P, :])
                    for m in range(MT):
                        pt = ps_t.tile([P, P], f32, tag="t")
                        nc.tensor.transpose(pt, st[:, m * P:(m + 1) * P], ident)
                        nc.vector.tensor_copy(
                            out=memT[m][:, jt * P:(jt + 1) * P], in_=pt)
                # transpose fk_w, fv_w
                for w_ap, wT in ((fkw_d, fkT), (fvw_d, fvT)):
                    for dt in range(DT):
                        st = p0st.tile([P, MD], f32, tag="st")
                        nc.sync.dma_start(out=st, in_=w_ap[dt * P:(dt + 1) * P, :])
                        for m in range(MT):
                            pt = ps_t.tile([P, P], f32, tag="t")
                            nc.tensor.transpose(pt, st[:, m * P:(m + 1) * P], ident)
                            nc.vector.tensor_copy(
                                out=wT[m][:, dt * P:(dt + 1) * P], in_=pt)

                # ekT[d-tile] = exp(sum_m fkT[m].T @ memT[m] + fk_b)
                for dt in range(DT):
                    for jc in range(J // 512):
                        ps = ps_s.tile([P, 512], f32, tag="s")
                        for m in range(MT):
                            nc.tensor.matmul(
                                ps,
                                lhsT=fkT[m][:, dt * P:(dt + 1) * P],
                                rhs=memT[m][:, jc * 512:(jc + 1) * 512],
                                start=(m == 0), stop=(m == MT - 1))
                        nc.scalar.activation(
                            out=ekT[dt][:, jc * 512:(jc + 1) * 512], in_=ps,
                            func=AF.Exp, bias=fkbT[:, dt:dt + 1], scale=1.0)

                # val[j-tile] = relu(sum_m memT[m].T @ fvT[m] + 1 x fv_b)
                for jt in range(JT):
                    for dh in range(D // 512):
                        ps = ps_s.tile([P, 512], f32, tag="s")
                        for m in range(MT):
                            nc.tensor.matmul(
                                ps,
                                lhsT=memT[m][:, jt * P:(jt + 1) * P],
                                rhs=fvT[m][:, dh * 512:(dh + 1) * 512],
                                start=(m == 0), stop=False)
                        nc.tensor.matmul(
                            ps, lhsT=ones_r16,
                            rhs=fvb16[:, dh * 512:(dh + 1) * 512],
                            start=False, stop=True)
                        nc.scalar.activation(
                            out=val[:, jt, dh * 512:(dh + 1) * 512], in_=ps,
                            func=AF.Relu)

            # key softmax denominators -> normalize ekT in place
            with tc.tile_pool(name="pc", bufs=2) as pc:
                cbc = pc.tile([P, J], bf16, tag="cbc")
                for jc in range(J // 512):
                    pd = ps_t.tile([1, 512], f32, tag="t")
                    for dt in range(DT):
                        nc.tensor.matmul(
                            pd, lhsT=ones_c16,
                            rhs=ekT[dt][:, jc * 512:(jc + 1) * 512],
                            start=(dt == 0), stop=(dt == DT - 1))
                    crow = pc.tile([1, 512], f32, tag="crow")
                    nc.vector.reciprocal(out=crow, in_=pd)
                    # broadcast across partitions via rank-1 fp32 matmul
                    po = ps_o.tile([P, 512], f32, tag="o")
                    nc.tensor.matmul(po, lhsT=ones_r32, rhs=crow,
                                     start=True, stop=True)
                    nc.vector.tensor_copy(
                        out=cbc[:, jc * 512:(jc + 1) * 512], in_=po)
                for dt in range(DT):
                    nc.vector.tensor_mul(ekT[dt], ekT[dt], cbc)

            # ---------------- Phase 1: attention over k rows ----------------
            with tc.tile_pool(name="p1", bufs=2) as p1, \
                 tc.tile_pool(name="p1e", bufs=1) as p1e:
                for ci in range(NC_):
                    n0 = ci * NCHUNK
                    # load + transpose k rows -> kT[d-tile, n] bf16
                    kT = p1.tile([P, DT, NCHUNK], bf16, tag="kT")
                    for ns in range(NS):
                        st = p1.tile([P, D], f32, tag="kst")
                        nc.sync.dma_start(
                            out=st, in_=k_d[n0 + ns * P:n0 + (ns + 1) * P, :])
                        for dc in range(DT):
                            pt = ps_t.tile([P, P], f32, tag="t")
                            nc.tensor.transpose(
                                pt, st[:, dc * P:(dc + 1) * P], ident)
                            nc.vector.tensor_copy(
                                out=kT[:, dc, ns * P:(ns + 1) * P], in_=pt)

                    # scoresT + exp -> E  (j on partitions, n free)
                    E = p1e.tile([P, JT, NCHUNK], bf16, tag="E")
                    for jp in range(JT // 2):
                        ps = ps_s.tile([P, 2, NCHUNK], f32, tag="s")
                        for h in range(2):
                            jt = jp * 2 + h
                            for dc in range(DT):
                                nc.tensor.matmul(
                                    ps[:, h, :],
                                    lhsT=ekT[dc][:, jt * P:(jt + 1) * P],
                                    rhs=kT[:, dc, :],
                                    start=(dc == 0), stop=(dc == DT - 1))
                        nc.scalar.activation(
                            out=E[:, jp * 2:jp * 2 + 2, :], in_=ps, func=AF.Exp)

                    # denom over j -> reciprocal -> per-partition layout
                    pd = ps_t.tile([1, NCHUNK], f32, tag="t")
                    for jt in range(JT):
                        nc.tensor.matmul(pd, lhsT=ones_c16, rhs=E[:, jt, :],
                                         start=(jt == 0), stop=(jt == JT - 1))
                    rrow = p1.tile([1, NCHUNK], f32, tag="rrow")
                    nc.vector.reciprocal(out=rrow, in_=pd)
                    rcol = p1.tile([P, NS], f32, tag="rcol")
                    for q in range(NS):
                        pt = ps_t.tile([P, 1], f32, tag="t")
                        nc.tensor.transpose(
                            pt, rrow[:, q * P:(q + 1) * P], ident[0:1, 0:1])
                        nc.vector.tensor_copy(out=rcol[:, q:q + 1], in_=pt)

                    # out[n-tile, dd] = (E.T @ val) * r
                    for ns in range(NS):
                        for dh in range(D // 512):
                            po = ps_o.tile([P, 512], f32, tag="o")
                            for jt in range(JT):
                                nc.tensor.matmul(
                                    po,
                                    lhsT=E[:, jt, ns * P:(ns + 1) * P],
                                    rhs=val[:, jt, dh * 512:(dh + 1) * 512],
                                    start=(jt == 0), stop=(jt == JT - 1))
                            osb = p1.tile([P, 512], f32, tag="osb")
                            nc.vector.tensor_scalar_mul(
                                osb, po, rcol[:, ns:ns + 1])
                            nc.sync.dma_start(
                                out=out_d[n0 + ns * P:n0 + (ns + 1) * P,
                                          dh * 512:(dh + 1) * 512],
                                in_=osb)

    nc.compile()
    return nc


def _get_nc():
    if "nc" not in _CACHE:
        _CACHE["nc"] = _build()
    return _CACHE["nc"]


def kernel(**inputs) -> np.ndarray:
    from concourse.bass_utils import run_bass_kernel_spmd

    k = np.ascontiguousarray(np.asarray(inputs["k"], dtype=np.float32))
    mem = np.ascontiguousarray(np.asarray(inputs["mem"], dtype=np.float32))
    fk_w = np.ascontiguousarray(np.asarray(inputs["fk_w"], dtype=np.float32))
    fk_b = np.ascontiguousarray(np.asarray(inputs["fk_b"], dtype=np.float32))
    fv_w = np.ascontiguousarray(np.asarray(inputs["fv_w"], dtype=np.float32))
    fv_b = np.ascontiguousarray(np.asarray(inputs["fv_b"], dtype=np.float32))
    ident = np.eye(P, dtype=np.float32)

    nc = _get_nc()
    in_maps = []
    for c in range(NCORES):
        in_maps.append({
            "k": k[c * S:(c + 1) * S],
            "mem": mem, "fk_w": fk_w, "fk_b": fk_b,
            "fv_w": fv_w, "fv_b": fv_b, "ident": ident,
        })
    res = run_bass_kernel_spmd(nc, in_maps, core_ids=list(range(NCORES)),
                               **_CACHE.get("run_kwargs", {}))
    _CACHE["last_result"] = res
    return np.concatenate([res.results[c]["out"] for c in range(NCORES)], axis=0)


# revision 6
# speedup vs baseline: 1.0152x; 1.0152x over previous
"""MemoryNet kernel for 8 TRN2 NeuronCores (Bass/Tile).

Reference (single-device):
    key = softmax(mem @ fk_w.T + fk_b, axis=-1)      # [J, D]
    val = relu(mem @ fv_w.T + fv_b)                  # [J, D]
    att = softmax(k @ key.T, axis=-1)                # [N, J]
    out = att @ val                                  # [N, D]
with J=4096 (num_mem), MD=512 (mem_dim), D=1024 (inp_dim), N=32768.

Sharding: data-parallel over rows of k (N) across 8 cores; mem + weights
replicated on every core. Each core computes out rows for its shard; the
host concatenates.

Per-core algorithm (big matmuls in bf16, fp32 accumulation, transposed
"scores" layout so softmax-over-j needs no on-chip transposes of att):
  Phase 0 (replicated derivation):
    memT[m,j], fk_wT[m,d], fv_wT[m,d] via PE transposes.
    ekT[d,j]  = exp(fk_wT.T @ memT + fk_b[d])    (unnormalized key^T; no max
                subtraction -- |logits| <= ~4 for this problem family)
    c[j]      = 1 / sum_d ekT   (ones-vector matmul over partitions +
                reciprocal), transposed into per-partition c_col[p, jt]
    val[j,dd] = relu(memT.T @ fv_wT + fv_b)  (fv_b added via rank-1 matmul)
  Phase 1 (per chunk of NCHUNK k-rows):
    kT[d,n]   via PE transposes of DMA'd k rows
    u[j-tile, n] = ekT.T @ kT        (PSUM accumulate over d)
    E = exp(c_j * u)                 (ACT with per-partition scale -> bf16)
    out_ps[n-tile, dd] = E.T @ val   (PSUM accumulate over j)
    den[n-tile, 1]     = E.T @ ones  (same lhsT, N=1 matmul)
    out = out_ps * (1/den)           (DVE reciprocal + tensor_scalar)
"""

import numpy as np

P = 128
J = 4096      # num_mem
MD = 512      # mem_dim
D = 1024      # inp_dim
NTOT = 32768  # total k rows
NCORES = 8
S = NTOT // NCORES   # k rows per core
NCHUNK = 512         # k rows processed per phase-1 chunk

_CACHE = {}


def _build():
    import concourse.bass as bass
    import concourse.tile as tile
    from concourse import bacc, mybir

    f32 = mybir.dt.float32
    bf16 = mybir.dt.bfloat16
    AF = mybir.ActivationFunctionType

    nc = bacc.Bacc("TRN2", target_bir_lowering=False, debug=False,
                   num_devices=NCORES)

    k_d = nc.dram_tensor("k", [S, D], f32, kind="ExternalInput").ap()
    mem_d = nc.dram_tensor("mem", [J, MD], f32, kind="ExternalInput").ap()
    fkw_d = nc.dram_tensor("fk_w", [D, MD], f32, kind="ExternalInput").ap()
    fkb_d = nc.dram_tensor("fk_b", [D], f32, kind="ExternalInput").ap()
    fvw_d = nc.dram_tensor("fv_w", [D, MD], f32, kind="ExternalInput").ap()
    fvb_d = nc.dram_tensor("fv_b", [D], f32, kind="ExternalInput").ap()
    id_d = nc.dram_tensor("ident", [P, P], f32, kind="ExternalInput").ap()
    out_d = nc.dram_tensor("out", [S, D], f32, kind="ExternalOutput").ap()

    JT = J // P        # 32 j-tiles
    DT = D // P        # 8 d-tiles
    MT = MD // P       # 4 m-tiles
    NC_ = S // NCHUNK  # phase-1 chunks
    NS = NCHUNK // P   # n-subtiles per chunk

    with tile.TileContext(nc) as tc:
        from contextlib import ExitStack
        ctx = ExitStack()
        with ctx:
            persist = ctx.enter_context(tc.tile_pool(name="persist", bufs=1))
            ps_s = ctx.enter_context(tc.tile_pool(name="ps_s", bufs=2, space="PSUM"))
            ps_o = ctx.enter_context(tc.tile_pool(name="ps_o", bufs=3, space="PSUM"))
            ps_d = ctx.enter_context(tc.tile_pool(name="ps_d", bufs=1, space="PSUM"))
            ps_t = ctx.enter_context(tc.tile_pool(name="ps_t", bufs=2, space="PSUM"))

            # persistent tiles
            ekT = [persist.tile([P, J], bf16, tag=f"ekT{d}", name=f"ekT{d}")
                   for d in range(DT)]
            val = persist.tile([P, JT, D], bf16, tag="val")
            ident = persist.tile([P, P], f32, tag="ident")
            ones_c16 = persist.tile([P, 1], bf16, tag="ones_c")   # colsum lhsT/rhs
            ones_r16 = persist.tile([1, P], bf16, tag="ones_r")   # rank-1 bias lhsT
            fkbT = persist.tile([P, DT], f32, tag="fkbT")
            c_col = persist.tile([P, JT], f32, tag="c_col")       # 1/keysum per j

            nc.sync.dma_start(out=ident, in_=id_d)
            nc.vector.memset(ones_c16, 1.0)
            nc.vector.memset(ones_r16, 1.0)
            # fk_b -> per-partition layout: fkbT[p, t] = fk_b[t*128 + p]
            nc.sync.dma_start(out=fkbT, in_=fkb_d.rearrange("(t p) -> p t", p=P))

            # ---------------- Phase 0: key/val derivation ----------------
            with tc.tile_pool(name="p0", bufs=1) as p0, \
                 tc.tile_pool(name="p0st", bufs=2) as p0st:
                memT = [p0.tile([P, J], bf16, tag=f"memT{m}", name=f"memT{m}")
                        for m in range(MT)]
                fkT = [p0.tile([P, D], bf16, tag=f"fkT{m}", name=f"fkT{m}")
                       for m in range(MT)]
                fvT = [p0.tile([P, D], bf16, tag=f"fvT{m}", name=f"fvT{m}")
                       for m in range(MT)]
                fvb16 = p0.tile([1, D], bf16, tag="fvb16")

                fvb32 = p0st.tile([1, D], f32, tag="fvb32")
                nc.sync.dma_start(out=fvb32,
                                  in_=fvb_d.rearrange("(a d) -> a d", a=1))
                nc.vector.tensor_copy(out=fvb16, in_=fvb32)

                # transpose mem -> memT (bf16)
                for jt in range(JT):
                    st = p0st.tile([P, MD], f32, tag="st")
                    nc.sync.dma_start(out=st, in_=mem_d[jt * P:(jt + 1) * P, :])
                    for m in range(MT):
                        pt = ps_t.tile([P, P], f32, tag="t")
                        nc.tensor.transpose(pt, st[:, m * P:(m + 1) * P], ident)
                        nc.vector.tensor_copy(
                            out=memT[m][:, jt * P:(jt + 1) * P], in_=pt)
                # transpose fk_w, fv_w
                for w_ap, wT in ((fkw_d, fkT), (fvw_d, fvT)):
                    for dt in range(DT):
                        st = p0st.tile([P, MD], f32, tag="st")
                        nc.sync.dma_start(out=st,
                                          in_=w_ap[dt * P:(dt + 1) * P, :])
                        for m in range(MT):
                            pt = ps_t.tile([P, P], f32, tag="t")
                            nc.tensor.transpose(pt, st[:, m * P:(m + 1) * P],
                                                ident)
                            nc.vector.tensor_copy(
                                out=wT[m][:, dt * P:(dt + 1) * P], in_=pt)

                # ekT[d-tile] = exp(sum_m fkT[m].T @ memT[m] + fk_b)
                for dt in range(DT):
                    for jc in range(J // 512):
                        ps = ps_s.tile([P, 512], f32, tag="s")
                        for m in range(MT):
                            nc.tensor.matmul(
                                ps,
                                lhsT=fkT[m][:, dt * P:(dt + 1) * P],
                                rhs=memT[m][:, jc * 512:(jc + 1) * 512],
                                start=(m == 0), stop=(m == MT - 1))
                        nc.scalar.activation(
                            out=ekT[dt][:, jc * 512:(jc + 1) * 512], in_=ps,
                            func=AF.Exp, bias=fkbT[:, dt:dt + 1], scale=1.0)

                # val[j-tile] = relu(sum_m memT[m].T @ fvT[m] + 1 x fv_b)
                for jt in range(JT):
                    for dh in range(D // 512):
                        ps = ps_s.tile([P, 512], f32, tag="s")
                        for m in range(MT):
                            nc.tensor.matmul(
                                ps,
                                lhsT=memT[m][:, jt * P:(jt + 1) * P],
                                rhs=fvT[m][:, dh * 512:(dh + 1) * 512],
                                start=(m == 0), stop=False)
                        nc.tensor.matmul(
                            ps, lhsT=ones_r16,
                            rhs=fvb16[:, dh * 512:(dh + 1) * 512],
                            start=False, stop=True)
                        nc.scalar.activation(
                            out=val[:, jt, dh * 512:(dh + 1) * 512], in_=ps,
                            func=AF.Relu)

                # key softmax denominators: c_col[p, jt] = 1/sum_d ekT[:, j]
                for jc in range(J // 512):
                    pd = ps_t.tile([1, 512], f32, tag="t")
                    for dt in range(DT):
                        nc.tensor.matmul(
                            pd, lhsT=ones_c16,
                            rhs=ekT[dt][:, jc * 512:(jc + 1) * 512],
                            start=(dt == 0), stop=(dt == DT - 1))
                    crow = p0st.tile([1, 512], f32, tag="crow")
                    nc.vector.reciprocal(out=crow, in_=pd)
                    for q in range(4):
                        pt = ps_t.tile([P, 1], f32, tag="t")
                        nc.tensor.transpose(pt, crow[:, q * P:(q + 1) * P],
                                            ident[0:1, 0:1])
                        jt = jc * 4 + q
                        nc.vector.tensor_copy(out=c_col[:, jt:jt + 1], in_=pt)

            # ---------------- Phase 1: attention over k rows ----------------
            with tc.tile_pool(name="p1", bufs=2) as p1, \
                 tc.tile_pool(name="p1e", bufs=1) as p1e:
                for ci in range(NC_):
                    n0 = ci * NCHUNK
                    # load + transpose k rows -> kT[d-tile, n] bf16
                    kT = p1.tile([P, DT, NCHUNK], bf16, tag="kT")
                    for ns in range(NS):
                        st = p1.tile([P, D], f32, tag="kst")
                        nc.sync.dma_start(
                            out=st, in_=k_d[n0 + ns * P:n0 + (ns + 1) * P, :])
                        for dc in range(DT):
                            pt = ps_t.tile([P, P], f32, tag="t")
                            nc.tensor.transpose(
                                pt, st[:, dc * P:(dc + 1) * P], ident)
                            nc.vector.tensor_copy(
                                out=kT[:, dc, ns * P:(ns + 1) * P], in_=pt)

                    # scoresT -> E = exp(c_j * u)   (j on partitions, n free)
                    E = p1e.tile([P, JT, NCHUNK], bf16, tag="E")
                    for jt in range(JT):
                        ps = ps_s.tile([P, NCHUNK], f32, tag="s")
                        for dc in range(DT):
                            nc.tensor.matmul(
                                ps,
                                lhsT=ekT[dc][:, jt * P:(jt + 1) * P],
                                rhs=kT[:, dc, :],
                                start=(dc == 0), stop=(dc == DT - 1))
                        nc.scalar.activation(
                            out=E[:, jt, :], in_=ps, func=AF.Exp,
                            scale=c_col[:, jt:jt + 1])

                    # out[n-tile, dd] = (E.T @ val) / (E.T @ 1)
                    for ns in range(NS):
                        po0 = ps_o.tile([P, 512], f32, tag="o")
                        po1 = ps_o.tile([P, 512], f32, tag="o")
                        pden = ps_d.tile([P, 1], f32, tag="den")
                        for jt in range(JT):
                            lhs = E[:, jt, ns * P:(ns + 1) * P]
                            st_, sp_ = (jt == 0), (jt == JT - 1)
                            nc.tensor.matmul(po0, lhsT=lhs,
                                             rhs=val[:, jt, 0:512],
                                             start=st_, stop=sp_)
                            nc.tensor.matmul(po1, lhsT=lhs,
                                             rhs=val[:, jt, 512:1024],
                                             start=st_, stop=sp_)
                            nc.tensor.matmul(pden, lhsT=lhs, rhs=ones_c16,
                                             start=st_, stop=sp_)
                        rv = p1.tile([P, 1], f32, tag="rv")
                        nc.vector.reciprocal(out=rv, in_=pden)
                        for dh, po in ((0, po0), (1, po1)):
                            osb = p1.tile([P, 512], f32, tag="osb")
                            nc.vector.tensor_scalar_mul(osb, po, rv)
                            nc.sync.dma_start(
                                out=out_d[n0 + ns * P:n0 + (ns + 1) * P,
                                          dh * 512:(dh + 1) * 512],
                                in_=osb)

    nc.compile()
    return nc


def _get_nc():
    if "nc" not in _CACHE:
        _CACHE["nc"] = _build()
    return _CACHE["nc"]


def kernel(**inputs) -> np.ndarray:
    from concourse.bass_utils import run_bass_kernel_spmd

    k = np.ascontiguousarray(np.asarray(inputs["k"], dtype=np.float32))
    mem = np.ascontiguousarray(np.asarray(inputs["mem"], dtype=np.float32))
    fk_w = np.ascontiguousarray(np.asarray(inputs["fk_w"], dtype=np.float32))
    fk_b = np.ascontiguousarray(np.asarray(inputs["fk_b"], dtype=np.float32))
    fv_w = np.ascontiguousarray(np.asarray(inputs["fv_w"], dtype=np.float32))
    fv_b = np.ascontiguousarray(np.asarray(inputs["fv_b"], dtype=np.float32))
    ident = np.eye(P, dtype=np.float32)

    nc = _get_nc()
    in_maps = []
    for c in range(NCORES):
        in_maps.append({
            "k": k[c * S:(c + 1) * S],
            "mem": mem, "fk_w": fk_w, "fk_b": fk_b,
            "fv_w": fv_w, "fv_b": fv_b, "ident": ident,
        })
    res = run_bass_kernel_spmd(nc, in_maps, core_ids=list(range(NCORES)),
                               **_CACHE.get("run_kwargs", {}))
    _CACHE["last_result"] = res
    return np.concatenate([res.results[c]["out"] for c in range(NCORES)],
                          axis=0)


# revision 13
# speedup vs baseline: 1.1236x; 1.1068x over previous
"""MemoryNet kernel for 8 TRN2 NeuronCores (Bass/Tile).

Reference (single-device):
    key = softmax(mem @ fk_w.T + fk_b, axis=-1)      # [J, D]
    val = relu(mem @ fv_w.T + fv_b)                  # [J, D]
    att = softmax(k @ key.T, axis=-1)                # [N, J]
    out = att @ val                                  # [N, D]
with J=4096 (num_mem), MD=512 (mem_dim), D=1024 (inp_dim), N=32768.

Sharding: data-parallel over rows of k (N) across 8 cores; mem + weights
replicated on every core. Each core computes out rows for its shard; the
host concatenates.

Per-core algorithm (big matmuls in bf16, fp32 accumulation, transposed
"scores" layout so softmax-over-j needs no on-chip transposes of att):
  Phase 0 (replicated derivation):
    memT[m,j], fk_wT[m,d], fv_wT[m,d] via PE transposes.
    ekT[d,j]  = exp(fk_wT.T @ memT + fk_b[d])    (unnormalized key^T; no max
                subtraction -- |logits| <= ~4 for this problem family)
    c[j]      = 1 / sum_d ekT   (ones-vector matmul over partitions +
                reciprocal), transposed into per-partition c_col[p, jt]
    val[j,dd] = relu(memT.T @ fv_wT + fv_b)  (fv_b added via rank-1 matmul)
  Phase 1 (per chunk of NCHUNK k-rows):
    kT[d,n]   via PE transposes of DMA'd k rows
    u[j-tile, n] = ekT.T @ kT        (PSUM accumulate over d)
    E = exp(c_j * u)                 (ACT with per-partition scale -> bf16)
    out_ps[n-tile, dd] = E.T @ val   (PSUM accumulate over j)
    den[n-tile, 1]     = E.T @ ones  (same lhsT, N=1 matmul)
    out = out_ps * (1/den)           (DVE reciprocal + tensor_scalar)
"""

import numpy as np

P = 128
J = 4096      # num_mem
MD = 512      # mem_dim
D = 1024      # inp_dim
NTOT = 32768  # total k rows
NCORES = 8
S = NTOT // NCORES   # k rows per core
NCHUNK = 512         # k rows processed per phase-1 chunk

_CACHE = {}


def _build():
    import concourse.bass as bass
    import concourse.tile as tile
    from concourse import bacc, mybir

    f32 = mybir.dt.float32
    bf16 = mybir.dt.bfloat16
    AF = mybir.ActivationFunctionType

    nc = bacc.Bacc("TRN2", target_bir_lowering=False, debug=False,
                   num_devices=NCORES)

    k_d = nc.dram_tensor("k", [S, D], f32, kind="ExternalInput").ap()
    mem_d = nc.dram_tensor("mem", [J, MD], f32, kind="ExternalInput").ap()
    fkw_d = nc.dram_tensor("fk_w", [D, MD], f32, kind="ExternalInput").ap()
    fkb_d = nc.dram_tensor("fk_b", [D], f32, kind="ExternalInput").ap()
    fvw_d = nc.dram_tensor("fv_w", [D, MD], f32, kind="ExternalInput").ap()
    fvb_d = nc.dram_tensor("fv_b", [D], f32, kind="ExternalInput").ap()
    id_d = nc.dram_tensor("ident", [P, P], f32, kind="ExternalInput").ap()
    out_d = nc.dram_tensor("out", [S, D], f32, kind="ExternalOutput").ap()

    JT = J // P        # 32 j-tiles
    DT = D // P        # 8 d-tiles
    MT = MD // P       # 4 m-tiles
    NC_ = S // NCHUNK  # phase-1 chunks
    NS = NCHUNK // P   # n-subtiles per chunk

    with tile.TileContext(nc) as tc:
        from contextlib import ExitStack
        ctx = ExitStack()
        with ctx:
            persist = ctx.enter_context(tc.tile_pool(name="persist", bufs=1))
            ps_s = ctx.enter_context(tc.tile_pool(name="ps_s", bufs=2, space="PSUM"))
            ps_o = ctx.enter_context(tc.tile_pool(name="ps_o", bufs=3, space="PSUM"))
            ps_d = ctx.enter_context(tc.tile_pool(name="ps_d", bufs=1, space="PSUM"))
            ps_t = ctx.enter_context(tc.tile_pool(name="ps_t", bufs=2, space="PSUM"))

            # persistent tiles
            ekT = [persist.tile([P, J], bf16, tag=f"ekT{d}", name=f"ekT{d}")
                   for d in range(DT)]
            val = persist.tile([P, JT, D], bf16, tag="val")
            ident = persist.tile([P, P], f32, tag="ident")
            ident16 = persist.tile([P, P], bf16, tag="ident16")
            ones_c16 = persist.tile([P, 1], bf16, tag="ones_c")   # colsum lhsT/rhs
            ones_r16 = persist.tile([1, P], bf16, tag="ones_r")   # rank-1 bias lhsT
            fkbT = persist.tile([P, DT], f32, tag="fkbT")
            c_col = persist.tile([P, JT], f32, tag="c_col")       # 1/keysum per j

            nc.sync.dma_start(out=ident, in_=id_d)
            nc.vector.tensor_copy(out=ident16, in_=ident)
            nc.vector.memset(ones_c16, 1.0)
            nc.vector.memset(ones_r16, 1.0)
            # fk_b -> per-partition layout: fkbT[p, t] = fk_b[t*128 + p]
            nc.sync.dma_start(out=fkbT, in_=fkb_d.rearrange("(t p) -> p t", p=P))

            # ---------------- Phase 0: key/val derivation ----------------
            with tc.tile_pool(name="p0", bufs=1) as p0, \
                 tc.tile_pool(name="p0st", bufs=4) as p0st:
                memT = [p0.tile([P, J], bf16, tag=f"memT{m}", name=f"memT{m}")
                        for m in range(MT)]
                fkT = [p0.tile([P, D], bf16, tag=f"fkT{m}", name=f"fkT{m}")
                       for m in range(MT)]
                fvT = [p0.tile([P, D], bf16, tag=f"fvT{m}", name=f"fvT{m}")
                       for m in range(MT)]
                fvb16 = p0.tile([1, D], bf16, tag="fvb16")

                fvb32 = p0st.tile([1, D], f32, tag="fvb32", bufs=1)
                nc.sync.dma_start(out=fvb32,
                                  in_=fvb_d.rearrange("(a d) -> a d", a=1))
                nc.vector.tensor_copy(out=fvb16, in_=fvb32)

                # transpose mem -> memT (cast to bf16 first: bf16 PE
                # transposes are ~2x cheaper than fp32 LOW/HIGH pairs)
                for jt in range(JT):
                    st = p0st.tile([P, MD], f32, tag="st")
                    nc.sync.dma_start(out=st, in_=mem_d[jt * P:(jt + 1) * P, :])
                    st16 = p0st.tile([P, MD], bf16, tag="st16", bufs=2)
                    nc.vector.tensor_copy(out=st16, in_=st)
                    for m in range(MT):
                        pt = ps_t.tile([P, P], bf16, tag="t")
                        nc.tensor.transpose(pt, st16[:, m * P:(m + 1) * P],
                                            ident16)
                        nc.vector.tensor_copy(
                            out=memT[m][:, jt * P:(jt + 1) * P], in_=pt)
                # transpose fk_w, fv_w
                for w_ap, wT in ((fkw_d, fkT), (fvw_d, fvT)):
                    for dt in range(DT):
                        st = p0st.tile([P, MD], f32, tag="st")
                        nc.sync.dma_start(out=st,
                                          in_=w_ap[dt * P:(dt + 1) * P, :])
                        st16 = p0st.tile([P, MD], bf16, tag="st16", bufs=2)
                        nc.vector.tensor_copy(out=st16, in_=st)
                        for m in range(MT):
                            pt = ps_t.tile([P, P], bf16, tag="t")
                            nc.tensor.transpose(pt, st16[:, m * P:(m + 1) * P],
                                                ident16)
                            nc.vector.tensor_copy(
                                out=wT[m][:, dt * P:(dt + 1) * P], in_=pt)

                # ekT[d-tile] = exp(sum_m fkT[m].T @ memT[m] + fk_b)
                for dt in range(DT):
                    for jc in range(J // 512):
                        ps = ps_s.tile([P, 512], f32, tag="s")
                        for m in range(MT):
                            nc.tensor.matmul(
                                ps,
                                lhsT=fkT[m][:, dt * P:(dt + 1) * P],
                                rhs=memT[m][:, jc * 512:(jc + 1) * 512],
                                start=(m == 0), stop=(m == MT - 1))
                        nc.scalar.activation(
                            out=ekT[dt][:, jc * 512:(jc + 1) * 512], in_=ps,
                            func=AF.Exp, bias=fkbT[:, dt:dt + 1], scale=1.0)

                # val[j-tile] = relu(sum_m memT[m].T @ fvT[m] + 1 x fv_b)
                for jt in range(JT):
                    for dh in range(D // 512):
                        ps = ps_s.tile([P, 512], f32, tag="s")
                        for m in range(MT):
                            nc.tensor.matmul(
                                ps,
                                lhsT=memT[m][:, jt * P:(jt + 1) * P],
                                rhs=fvT[m][:, dh * 512:(dh + 1) * 512],
                                start=(m == 0), stop=False)
                        nc.tensor.matmul(
                            ps, lhsT=ones_r16,
                            rhs=fvb16[:, dh * 512:(dh + 1) * 512],
                            start=False, stop=True)
                        nc.scalar.activation(
                            out=val[:, jt, dh * 512:(dh + 1) * 512], in_=ps,
                            func=AF.Relu)

                # key softmax denominators: c_col[p, jt] = 1/sum_d ekT[:, j]
                # (transpose the [1,512] row FIRST, then one [128,4]
                # reciprocal -- a [1,512] DVE reciprocal costs 3.3us serial)
                for jc in range(J // 512):
                    pd = ps_t.tile([1, 512], f32, tag="t")
                    for dt in range(DT):
                        nc.tensor.matmul(
                            pd, lhsT=ones_c16,
                            rhs=ekT[dt][:, jc * 512:(jc + 1) * 512],
                            start=(dt == 0), stop=(dt == DT - 1))
                    crow = p0st.tile([1, 512], f32, tag="crow", bufs=2)
                    nc.vector.tensor_copy(out=crow, in_=pd)
                    pq = ps_t.tile([P, 4], f32, tag="t")
                    for q in range(4):
                        nc.tensor.transpose(pq[:, q:q + 1],
                                            crow[:, q * P:(q + 1) * P],
                                            ident[0:1, 0:1])
                    nc.vector.reciprocal(
                        out=c_col[:, jc * 4:(jc + 1) * 4], in_=pq)

            # ---------------- Phase 1: attention over k rows ----------------
            with tc.tile_pool(name="p1", bufs=2) as p1, \
                 tc.tile_pool(name="p1e", bufs=1) as p1e:
                for ci in range(NC_):
                    n0 = ci * NCHUNK
                    # load + transpose k rows -> kT[d-tile, n] bf16
                    kT = p1.tile([P, DT, NCHUNK], bf16, tag="kT", bufs=1)
                    for ns in range(NS):
                        st = p1.tile([P, D], f32, tag="kst")
                        nc.sync.dma_start(
                            out=st, in_=k_d[n0 + ns * P:n0 + (ns + 1) * P, :])
                        st16 = p1.tile([P, D], bf16, tag="kst16")
                        nc.vector.tensor_copy(out=st16, in_=st)
                        for dc in range(DT):
                            pt = ps_t.tile([P, P], bf16, tag="t")
                            nc.tensor.transpose(
                                pt, st16[:, dc * P:(dc + 1) * P], ident16)
                            nc.vector.tensor_copy(
                                out=kT[:, dc, ns * P:(ns + 1) * P], in_=pt)

                    # scoresT -> E = exp(c_j * u)   (j on partitions, n free)
                    E = p1e.tile([P, JT, NCHUNK], bf16, tag="E")
                    for jt in range(JT):
                        ps = ps_s.tile([P, NCHUNK], f32, tag="s")
                        for dc in range(DT):
                            nc.tensor.matmul(
                                ps,
                                lhsT=ekT[dc][:, jt * P:(jt + 1) * P],
                                rhs=kT[:, dc, :],
                                start=(dc == 0), stop=(dc == DT - 1))
                        nc.scalar.activation(
                            out=E[:, jt, :], in_=ps, func=AF.Exp,
                            scale=c_col[:, jt:jt + 1])

                    # out[n-tile, dd] = (E.T @ val) / (E.T @ 1)
                    for ns in range(NS):
                        po0 = ps_o.tile([P, 512], f32, tag="o")
                        po1 = ps_o.tile([P, 512], f32, tag="o")
                        pden = ps_d.tile([P, 1], f32, tag="den")
                        for jt in range(JT):
                            lhs = E[:, jt, ns * P:(ns + 1) * P]
                            st_, sp_ = (jt == 0), (jt == JT - 1)
                            nc.tensor.matmul(po0, lhsT=lhs,
                                             rhs=val[:, jt, 0:512],
                                             start=st_, stop=sp_)
                            nc.tensor.matmul(po1, lhsT=lhs,
                                             rhs=val[:, jt, 512:1024],
                                             start=st_, stop=sp_)
                            nc.tensor.matmul(pden, lhsT=lhs, rhs=ones_c16,
                                             start=st_, stop=sp_)
                        rv = p1.tile([P, 1], f32, tag="rv")
                        nc.vector.reciprocal(out=rv, in_=pden)
                        for dh, po in ((0, po0), (1, po1)):
                            osb = p1.tile([P, 512], f32, tag="osb")
                            nc.vector.tensor_scalar_mul(osb, po, rv)
                            nc.sync.dma_start(
                                out=out_d[n0 + ns * P:n0 + (ns + 1) * P,
                                          dh * 512:(dh + 1) * 512],
                                in_=osb)

    nc.compile()
    return nc


def _get_nc():
    if "nc" not in _CACHE:
        _CACHE["nc"] = _build()
    return _CACHE["nc"]


def kernel(**inputs) -> np.ndarray:
    from concourse.bass_utils import run_bass_kernel_spmd

    k = np.ascontiguousarray(np.asarray(inputs["k"], dtype=np.float32))
    mem = np.ascontiguousarray(np.asarray(inputs["mem"], dtype=np.float32))
    fk_w = np.ascontiguousarray(np.asarray(inputs["fk_w"], dtype=np.float32))
    fk_b = np.ascontiguousarray(np.asarray(inputs["fk_b"], dtype=np.float32))
    fv_w = np.ascontiguousarray(np.asarray(inputs["fv_w"], dtype=np.float32))
    fv_b = np.ascontiguousarray(np.asarray(inputs["fv_b"], dtype=np.float32))
    ident = np.eye(P, dtype=np.float32)

    nc = _get_nc()
    in_maps = []
    for c in range(NCORES):
        in_maps.append({
            "k": k[c * S:(c + 1) * S],
            "mem": mem, "fk_w": fk_w, "fk_b": fk_b,
            "fv_w": fv_w, "fv_b": fv_b, "ident": ident,
        })
    res = run_bass_kernel_spmd(nc, in_maps, core_ids=list(range(NCORES)),
                               **_CACHE.get("run_kwargs", {}))
    _CACHE["last_result"] = res
    return np.concatenate([res.results[c]["out"] for c in range(NCORES)],
                          axis=0)


# revision 16
# speedup vs baseline: 1.5086x; 1.3427x over previous
"""MemoryNet kernel for 8 TRN2 NeuronCores (Bass/Tile).

Reference (single-device):
    key = softmax(mem @ fk_w.T + fk_b, axis=-1)      # [J, D]
    val = relu(mem @ fv_w.T + fv_b)                  # [J, D]
    att = softmax(k @ key.T, axis=-1)                # [N, J]
    out = att @ val                                  # [N, D]
with J=4096 (num_mem), MD=512 (mem_dim), D=1024 (inp_dim), N=32768.

Sharding: data-parallel over rows of k (N) across 8 cores; mem + weights
replicated on every core. Each core computes out rows for its shard; the
host concatenates.

Per-core algorithm. Derivation matmuls in bf16; the two big attention
matmuls run in fp8e4m3 with perf_mode=DoubleRow (2 contraction rows per
PE cell). fp8's ~6% relative steps would destroy att's small softmax
deviations if E=exp(s)~1.0 were quantized directly, so we store
Es = E - 1 (|Es|~0.04, 12x better absolute precision) and reconstruct:
    out = (colsum(val) + Es @ val) / (J + Es @ 1)
which matches full-bf16 accuracy (~6e-4 scale-relative, measured).

  Phase 0 (replicated derivation):
    memT/fk_wT/fv_wT via bf16 PE transposes.
    ekT[d,j]  = exp(fk_wT.T @ memT + fk_b[d])  -> fp8, d-pair-interleaved
    c[j]      = 1 / sum_d ekT   (ones matmul + transposed reciprocal)
    val[j,dd] = relu(memT.T @ fv_wT + fv_b)    -> fp8 (+ bf16 temp for
                valsum), fv_b added via rank-1 matmul
    valsum    = colsum(val)  (fp32 psum), broadcast to [128, D]
  Phase 1 (per chunk of NCHUNK k-rows):
    kT8[d,n]  via fp8 PE transposes of DMA'd k rows
    u[j-tile, n] = ekT8.T @ kT8      (DoubleRow, PSUM accumulate over d)
    exp in-place on PSUM (ACT, per-partition scale c_j), then
    Es = u - 1 -> fp8 SBUF (DVE)
    num[n-tile, dd] = Es.T @ val     (DoubleRow, accumulate over j)
    den[n-tile, 1]  = Es.T @ ones    (same lhsT, free-dim-1 matmul)
    out = (num + valsum) * 1/(J + den)
"""

import numpy as np

P = 128
J = 4096      # num_mem
MD = 512      # mem_dim
D = 1024      # inp_dim
NTOT = 32768  # total k rows
NCORES = 8
S = NTOT // NCORES   # k rows per core
NCHUNK = 512         # k rows processed per phase-1 chunk

_CACHE = {}


def _build():
    import concourse.bass as bass
    import concourse.tile as tile
    from concourse import bacc, mybir

    f32 = mybir.dt.float32
    bf16 = mybir.dt.bfloat16
    fp8 = mybir.dt.float8e4
    DR = mybir.MatmulPerfMode.DoubleRow
    AF = mybir.ActivationFunctionType

    nc = bacc.Bacc("TRN2", target_bir_lowering=False, debug=False,
                   num_devices=NCORES)

    k_d = nc.dram_tensor("k", [S, D], f32, kind="ExternalInput").ap()
    mem_d = nc.dram_tensor("mem", [J, MD], f32, kind="ExternalInput").ap()
    fkw_d = nc.dram_tensor("fk_w", [D, MD], f32, kind="ExternalInput").ap()
    fkb_d = nc.dram_tensor("fk_b", [D], f32, kind="ExternalInput").ap()
    fvw_d = nc.dram_tensor("fv_w", [D, MD], f32, kind="ExternalInput").ap()
    fvb_d = nc.dram_tensor("fv_b", [D], f32, kind="ExternalInput").ap()
    id_d = nc.dram_tensor("ident", [P, P], f32, kind="ExternalInput").ap()
    out_d = nc.dram_tensor("out", [S, D], f32, kind="ExternalOutput").ap()

    JT = J // P        # 32 j-tiles
    DT = D // P        # 8 d-tiles
    MT = MD // P       # 4 m-tiles
    NC_ = S // NCHUNK  # phase-1 chunks
    NS = NCHUNK // P   # n-subtiles per chunk

    with tile.TileContext(nc) as tc:
        from contextlib import ExitStack
        ctx = ExitStack()
        with ctx:
            persist = ctx.enter_context(tc.tile_pool(name="persist", bufs=1))
            ps_s = ctx.enter_context(tc.tile_pool(name="ps_s", bufs=3, space="PSUM"))
            ps_d = ctx.enter_context(tc.tile_pool(name="ps_d", bufs=1, space="PSUM"))
            ps_t = ctx.enter_context(tc.tile_pool(name="ps_t", bufs=1, space="PSUM"))

            # persistent tiles.  fp8 operands for DoubleRow matmuls are laid
            # out pair-interleaved: plane [.., i2, o, ..] holds contraction
            # row 256*i2 + 128*o + p.
            ekT8 = persist.tile([P, DT // 2, 2, J], fp8, tag="ekT8")
            val8 = persist.tile([P, JT // 2, 2, D], fp8, tag="val8")
            vsum_bc = persist.tile([P, D], f32, tag="vsum_bc")
            ident = persist.tile([P, P], f32, tag="ident")
            ident16 = persist.tile([P, P], bf16, tag="ident16")
            ident8 = persist.tile([P, P], fp8, tag="ident8")
            ones_c16 = persist.tile([P, 1], bf16, tag="ones_c")   # colsum lhsT
            ones_c8 = persist.tile([P, 1], fp8, tag="ones_c8")    # fp8 colsum lhsT
            ones8 = persist.tile([P, 2, 16], fp8, tag="ones8")    # DR den rhs
            ones_r16 = persist.tile([1, P], bf16, tag="ones_r")   # rank-1 bias lhsT
            ones_r32 = persist.tile([1, P], f32, tag="ones_r32")  # rank-1 f32 lhsT
            fkbT = persist.tile([P, DT], f32, tag="fkbT")
            c_col = persist.tile([P, JT], f32, tag="c_col")       # 1/keysum per j

            nc.sync.dma_start(out=ident, in_=id_d)
            nc.vector.tensor_copy(out=ident16, in_=ident)
            nc.vector.tensor_copy(out=ident8, in_=ident)
            nc.vector.memset(ones_c16, 1.0)
            nc.vector.memset(ones_c8, 1.0)
            nc.vector.memset(ones8, 1.0)
            nc.vector.memset(ones_r16, 1.0)
            nc.vector.memset(ones_r32, 1.0)
            # fk_b -> per-partition layout: fkbT[p, t] = fk_b[t*128 + p]
            nc.sync.dma_start(out=fkbT, in_=fkb_d.rearrange("(t p) -> p t", p=P))

            # ---------------- Phase 0: key/val derivation ----------------
            with tc.tile_pool(name="p0", bufs=1) as p0, \
                 tc.tile_pool(name="p0st", bufs=4) as p0st, \
                 tc.tile_pool(name="ps_vs", bufs=2, space="PSUM") as ps_vs:
                memT = [p0.tile([P, J], bf16, tag=f"memT{m}", name=f"memT{m}")
                        for m in range(MT)]
                fkT = [p0.tile([P, D], bf16, tag=f"fkT{m}", name=f"fkT{m}")
                       for m in range(MT)]
                fvT = [p0.tile([P, D], bf16, tag=f"fvT{m}", name=f"fvT{m}")
                       for m in range(MT)]
                fvb16 = p0.tile([1, D], bf16, tag="fvb16")

                fvb32 = p0st.tile([1, D], f32, tag="fvb32", bufs=1)
                nc.sync.dma_start(out=fvb32,
                                  in_=fvb_d.rearrange("(a d) -> a d", a=1))
                nc.vector.tensor_copy(out=fvb16, in_=fvb32)

                # transpose mem -> memT (cast to bf16 first: bf16 PE
                # transposes are ~2x cheaper than fp32 LOW/HIGH pairs)
                for jt in range(JT):
                    st = p0st.tile([P, MD], f32, tag="st")
                    nc.sync.dma_start(out=st, in_=mem_d[jt * P:(jt + 1) * P, :])
                    st16 = p0st.tile([P, MD], bf16, tag="st16", bufs=2)
                    nc.vector.tensor_copy(out=st16, in_=st)
                    for m in range(MT):
                        pt = ps_t.tile([P, P], bf16, tag="t")
                        nc.tensor.transpose(pt, st16[:, m * P:(m + 1) * P],
                                            ident16)
                        nc.vector.tensor_copy(
                            out=memT[m][:, jt * P:(jt + 1) * P], in_=pt)
                # transpose fk_w, fv_w
                for w_ap, wT in ((fkw_d, fkT), (fvw_d, fvT)):
                    for dt in range(DT):
                        st = p0st.tile([P, MD], f32, tag="st")
                        nc.sync.dma_start(out=st,
                                          in_=w_ap[dt * P:(dt + 1) * P, :])
                        st16 = p0st.tile([P, MD], bf16, tag="st16", bufs=2)
                        nc.vector.tensor_copy(out=st16, in_=st)
                        for m in range(MT):
                            pt = ps_t.tile([P, P], bf16, tag="t")
                            nc.tensor.transpose(pt, st16[:, m * P:(m + 1) * P],
                                                ident16)
                            nc.vector.tensor_copy(
                                out=wT[m][:, dt * P:(dt + 1) * P], in_=pt)

                # ekT8[d-tile] = exp(sum_m fkT[m].T @ memT[m] + fk_b), fp8
                for dt in range(DT):
                    for jc in range(J // 512):
                        ps = ps_s.tile([P, 512], f32, tag="s")
                        for m in range(MT):
                            nc.tensor.matmul(
                                ps,
                                lhsT=fkT[m][:, dt * P:(dt + 1) * P],
                                rhs=memT[m][:, jc * 512:(jc + 1) * 512],
                                start=(m == 0), stop=(m == MT - 1))
                        nc.scalar.activation(
                            out=ekT8[:, dt // 2, dt % 2,
                                     jc * 512:(jc + 1) * 512],
                            in_=ps, func=AF.Exp, bias=fkbT[:, dt:dt + 1],
                            scale=1.0)

                # val8[j-tile] = relu(sum_m memT[m].T @ fvT[m] + 1 x fv_b)
                # and valsum[dd] = colsum(val) accumulated in fp32 psum
                pv0 = ps_vs.tile([1, 512], f32, tag="vs")
                pv1 = ps_vs.tile([1, 512], f32, tag="vs")
                for jt in range(JT):
                    vt16 = p0st.tile([P, D], bf16, tag="vt16", bufs=2)
                    for dh in range(D // 512):
                        ps = ps_s.tile([P, 512], f32, tag="s")
                        for m in range(MT):
                            nc.tensor.matmul(
                                ps,
                                lhsT=memT[m][:, jt * P:(jt + 1) * P],
                                rhs=fvT[m][:, dh * 512:(dh + 1) * 512],
                                start=(m == 0), stop=False)
                        nc.tensor.matmul(
                            ps, lhsT=ones_r16,
                            rhs=fvb16[:, dh * 512:(dh + 1) * 512],
                            start=False, stop=True)
                        nc.scalar.activation(
                            out=vt16[:, dh * 512:(dh + 1) * 512], in_=ps,
                            func=AF.Relu)
                    nc.vector.tensor_copy(out=val8[:, jt // 2, jt % 2, :],
                                          in_=vt16)
                    nc.tensor.matmul(pv0, lhsT=ones_c16, rhs=vt16[:, 0:512],
                                     start=(jt == 0), stop=(jt == JT - 1))
                    nc.tensor.matmul(pv1, lhsT=ones_c16, rhs=vt16[:, 512:1024],
                                     start=(jt == 0), stop=(jt == JT - 1))
                # broadcast valsum across partitions (rank-1 fp32 matmul)
                vs_row = p0.tile([1, D], f32, tag="vs_row")
                nc.vector.tensor_copy(out=vs_row[:, 0:512], in_=pv0)
                nc.vector.tensor_copy(out=vs_row[:, 512:1024], in_=pv1)
                for dh in range(D // 512):
                    pb = ps_s.tile([P, 512], f32, tag="s")
                    nc.tensor.matmul(pb, lhsT=ones_r32,
                                     rhs=vs_row[:, dh * 512:(dh + 1) * 512],
                                     start=True, stop=True)
                    nc.vector.tensor_copy(
                        out=vsum_bc[:, dh * 512:(dh + 1) * 512], in_=pb)

                # key softmax denominators: c_col[p, jt] = 1/sum_d ekT[:, j]
                for jc in range(J // 512):
                    pd = ps_t.tile([1, 512], f32, tag="t")
                    for dt in range(DT):
                        nc.tensor.matmul(
                            pd, lhsT=ones_c8,
                            rhs=ekT8[:, dt // 2, dt % 2,
                                     jc * 512:(jc + 1) * 512],
                            start=(dt == 0), stop=(dt == DT - 1))
                    crow = p0st.tile([1, 512], f32, tag="crow", bufs=2)
                    nc.vector.tensor_copy(out=crow, in_=pd)
                    pq = ps_t.tile([P, 4], f32, tag="t")
                    for q in range(4):
                        nc.tensor.transpose(pq[:, q:q + 1],
                                            crow[:, q * P:(q + 1) * P],
                                            ident[0:1, 0:1])
                    nc.vector.reciprocal(
                        out=c_col[:, jc * 4:(jc + 1) * 4], in_=pq)

            # ---------------- Phase 1: attention over k rows ----------------
            with tc.tile_pool(name="p1", bufs=2) as p1, \
                 tc.tile_pool(name="p1e", bufs=2) as p1e, \
                 tc.tile_pool(name="ps_o", bufs=3, space="PSUM") as ps_o:
                for ci in range(NC_):
                    n0 = ci * NCHUNK
                    # load + cast + transpose k rows -> kT8[d-pair, n] fp8
                    kT8 = p1.tile([P, DT // 2, 2, NCHUNK], fp8, tag="kT8")
                    for ns in range(NS):
                        st = p1.tile([P, D], f32, tag="kst")
                        nc.sync.dma_start(
                            out=st, in_=k_d[n0 + ns * P:n0 + (ns + 1) * P, :])
                        st16 = p1.tile([P, D], bf16, tag="kst16")
                        nc.vector.tensor_copy(out=st16, in_=st)
                        for dc in range(DT):
                            pt = ps_t.tile([P, P], bf16, tag="t")
                            nc.tensor.transpose(
                                pt, st16[:, dc * P:(dc + 1) * P], ident16)
                            nc.vector.tensor_copy(
                                out=kT8[:, dc // 2, dc % 2,
                                        ns * P:(ns + 1) * P],
                                in_=pt)

                    # scoresT (DoubleRow) -> exp in place -> Es = E-1 (fp8)
                    Es8 = p1e.tile([P, JT // 2, 2, NCHUNK], fp8, tag="Es8")
                    for jt in range(JT):
                        ps = ps_s.tile([P, NCHUNK], f32, tag="s")
                        for dc2 in range(DT // 2):
                            nc.tensor.matmul(
                                ps,
                                lhsT=ekT8[:, dc2, :, jt * P:(jt + 1) * P],
                                rhs=kT8[:, dc2, :, :],
                                start=(dc2 == 0), stop=(dc2 == DT // 2 - 1),
                                perf_mode=DR)
                        nc.scalar.activation(
                            out=ps, in_=ps, func=AF.Exp,
                            scale=c_col[:, jt:jt + 1])
                        nc.vector.tensor_scalar_add(
                            Es8[:, jt // 2, jt % 2, :], ps, -1.0)

                    # out[n-tile, dd] = (vsum + Es.T @ val) / (J + Es.T @ 1)
                    for ns in range(NS):
                        po0 = ps_o.tile([P, 512], f32, tag="o")
                        po1 = ps_o.tile([P, 512], f32, tag="o")
                        pden = ps_d.tile([P, 1], f32, tag="den")
                        for jc2 in range(JT // 2):
                            lhs = Es8[:, jc2, :, ns * P:(ns + 1) * P]
                            st_, sp_ = (jc2 == 0), (jc2 == JT // 2 - 1)
                            nc.tensor.matmul(po0, lhsT=lhs,
                                             rhs=val8[:, jc2, :, 0:512],
                                             start=st_, stop=sp_, perf_mode=DR)
                            nc.tensor.matmul(po1, lhsT=lhs,
                                             rhs=val8[:, jc2, :, 512:1024],
                                             start=st_, stop=sp_, perf_mode=DR)
                            nc.tensor.matmul(pden, lhsT=lhs,
                                             rhs=ones8[:, :, 0:1],
                                             start=st_, stop=sp_, perf_mode=DR)
                        rv = p1.tile([P, 1], f32, tag="rv")
                        nc.vector.tensor_scalar_add(rv, pden, float(J))
                        nc.vector.reciprocal(out=rv, in_=rv)
                        for dh, po in ((0, po0), (1, po1)):
                            osb = p1.tile([P, 512], f32, tag="osb")
                            nc.vector.tensor_add(
                                osb, po, vsum_bc[:, dh * 512:(dh + 1) * 512])
                            nc.vector.tensor_scalar_mul(osb, osb, rv)
                            nc.sync.dma_start(
                                out=out_d[n0 + ns * P:n0 + (ns + 1) * P,
                                          dh * 512:(dh + 1) * 512],
                                in_=osb)

    nc.compile()
    return nc


def _get_nc():
    if "nc" not in _CACHE:
        _CACHE["nc"] = _build()
    return _CACHE["nc"]


def kernel(**inputs) -> np.ndarray:
    from concourse.bass_utils import run_bass_kernel_spmd

    k = np.ascontiguousarray(np.asarray(inputs["k"], dtype=np.float32))
    mem = np.ascontiguousarray(np.asarray(inputs["mem"], dtype=np.float32))
    fk_w = np.ascontiguousarray(np.asarray(inputs["fk_w"], dtype=np.float32))
    fk_b = np.ascontiguousarray(np.asarray(inputs["fk_b"], dtype=np.float32))
    fv_w = np.ascontiguousarray(np.asarray(inputs["fv_w"], dtype=np.float32))
    fv_b = np.ascontiguousarray(np.asarray(inputs["fv_b"], dtype=np.float32))
    ident = np.eye(P, dtype=np.float32)

    nc = _get_nc()
    in_maps = []
    for c in range(NCORES):
        in_maps.append({
            "k": k[c * S:(c + 1) * S],
            "mem": mem, "fk_w": fk_w, "fk_b": fk_b,
            "fv_w": fv_w, "fv_b": fv_b, "ident": ident,
        })
    res = run_bass_kernel_spmd(nc, in_maps, core_ids=list(range(NCORES)),
                               **_CACHE.get("run_kwargs", {}))
    _CACHE["last_result"] = res
    return np.concatenate([res.results[c]["out"] for c in range(NCORES)],
                          axis=0)


# revision 17
# speedup vs baseline: 1.7235x; 1.1425x over previous
"""MemoryNet kernel for 8 TRN2 NeuronCores (Bass/Tile).

Reference (single-device):
    key = softmax(mem @ fk_w.T + fk_b, axis=-1)      # [J, D]
    val = relu(mem @ fv_w.T + fv_b)                  # [J, D]
    att = softmax(k @ key.T, axis=-1)                # [N, J]
    out = att @ val                                  # [N, D]
with J=4096 (num_mem), MD=512 (mem_dim), D=1024 (inp_dim), N=32768.

Sharding: data-parallel over rows of k (N) across 8 cores; mem + weights
replicated on every core. Each core computes out rows for its shard; the
host concatenates.

Per-core algorithm. Derivation matmuls in bf16; the two big attention
matmuls run in fp8e4m3 with perf_mode=DoubleRow (2 contraction rows per
PE cell). fp8's ~6% relative steps would destroy att's small softmax
deviations if E=exp(s)~1.0 were quantized directly, so we store
Es = E - 1 (|Es|~0.04, 12x better absolute precision) and reconstruct:
    out = (colsum(val) + Es @ val) / (J + Es @ 1)
which matches full-bf16 accuracy (~6e-4 scale-relative, measured).

  Phase 0 (replicated derivation):
    memT/fk_wT/fv_wT via bf16 PE transposes.
    ekT[d,j]  = exp(fk_wT.T @ memT + fk_b[d])  -> fp8, d-pair-interleaved
    c[j]      = 1 / sum_d ekT   (ones matmul + transposed reciprocal)
    val[j,dd] = relu(memT.T @ fv_wT + fv_b)    -> fp8 (+ bf16 temp for
                valsum), fv_b added via rank-1 matmul
    valsum    = colsum(val)  (fp32 psum), broadcast to [128, D]
  Phase 1 (per chunk of NCHUNK k-rows):
    kT8[d,n]  via fp8 PE transposes of DMA'd k rows
    u[j-tile, n] = ekT8.T @ kT8      (DoubleRow, PSUM accumulate over d)
    exp in-place on PSUM (ACT, per-partition scale c_j), then
    Es = u - 1 -> fp8 SBUF (DVE)
    num[n-tile, dd] = Es.T @ val     (DoubleRow, accumulate over j)
    den[n-tile, 1]  = Es.T @ ones    (same lhsT, free-dim-1 matmul)
    out = (num + valsum) * 1/(J + den)
"""

import numpy as np

P = 128
J = 4096      # num_mem
MD = 512      # mem_dim
D = 1024      # inp_dim
NTOT = 32768  # total k rows
NCORES = 8
S = NTOT // NCORES   # k rows per core
NCHUNK = 512         # k rows processed per phase-1 chunk

_CACHE = {}


def _build():
    import concourse.bass as bass
    import concourse.tile as tile
    from concourse import bacc, mybir

    f32 = mybir.dt.float32
    bf16 = mybir.dt.bfloat16
    fp8 = mybir.dt.float8e4
    DR = mybir.MatmulPerfMode.DoubleRow
    AF = mybir.ActivationFunctionType

    nc = bacc.Bacc("TRN2", target_bir_lowering=False, debug=False,
                   num_devices=NCORES)

    k_d = nc.dram_tensor("k", [S, D], f32, kind="ExternalInput").ap()
    mem_d = nc.dram_tensor("mem", [J, MD], f32, kind="ExternalInput").ap()
    fkw_d = nc.dram_tensor("fk_w", [D, MD], f32, kind="ExternalInput").ap()
    fkb_d = nc.dram_tensor("fk_b", [D], f32, kind="ExternalInput").ap()
    fvw_d = nc.dram_tensor("fv_w", [D, MD], f32, kind="ExternalInput").ap()
    fvb_d = nc.dram_tensor("fv_b", [D], f32, kind="ExternalInput").ap()
    id_d = nc.dram_tensor("ident", [P, P], f32, kind="ExternalInput").ap()
    out_d = nc.dram_tensor("out", [S, D], f32, kind="ExternalOutput").ap()

    JT = J // P        # 32 j-tiles
    DT = D // P        # 8 d-tiles
    MT = MD // P       # 4 m-tiles
    NC_ = S // NCHUNK  # phase-1 chunks
    NS = NCHUNK // P   # n-subtiles per chunk

    with tile.TileContext(nc) as tc:
        from contextlib import ExitStack
        ctx = ExitStack()
        with ctx:
            persist = ctx.enter_context(tc.tile_pool(name="persist", bufs=1))
            ps_s = ctx.enter_context(tc.tile_pool(name="ps_s", bufs=4, space="PSUM"))
            ps_d = ctx.enter_context(tc.tile_pool(name="ps_d", bufs=1, space="PSUM"))
            ps_t = ctx.enter_context(tc.tile_pool(name="ps_t", bufs=1, space="PSUM"))

            # persistent tiles.  fp8 operands for DoubleRow matmuls are laid
            # out pair-interleaved: plane [.., i2, o, ..] holds contraction
            # row 256*i2 + 128*o + p.
            ekT8 = persist.tile([P, DT // 2, 2, J], fp8, tag="ekT8")
            val8 = persist.tile([P, JT // 2, 2, D], fp8, tag="val8")
            vsum_bc = persist.tile([P, D], f32, tag="vsum_bc")
            ident = persist.tile([P, P], f32, tag="ident")
            ident16 = persist.tile([P, P], bf16, tag="ident16")
            ident8 = persist.tile([P, P], fp8, tag="ident8")
            ones_c16 = persist.tile([P, 1], bf16, tag="ones_c")   # colsum lhsT
            ones_c8 = persist.tile([P, 1], fp8, tag="ones_c8")    # fp8 colsum lhsT
            ones8 = persist.tile([P, 2, 16], fp8, tag="ones8")    # DR den rhs
            ones_r16 = persist.tile([1, P], bf16, tag="ones_r")   # rank-1 bias lhsT
            ones_r32 = persist.tile([1, P], f32, tag="ones_r32")  # rank-1 f32 lhsT
            fkbT = persist.tile([P, DT], f32, tag="fkbT")
            c_col = persist.tile([P, JT], f32, tag="c_col")       # 1/keysum per j

            nc.sync.dma_start(out=ident, in_=id_d)
            nc.vector.tensor_copy(out=ident16, in_=ident)
            nc.vector.tensor_copy(out=ident8, in_=ident)
            nc.vector.memset(ones_c16, 1.0)
            nc.vector.memset(ones_c8, 1.0)
            nc.vector.memset(ones8, 1.0)
            nc.vector.memset(ones_r16, 1.0)
            nc.vector.memset(ones_r32, 1.0)
            # fk_b -> per-partition layout: fkbT[p, t] = fk_b[t*128 + p]
            nc.sync.dma_start(out=fkbT, in_=fkb_d.rearrange("(t p) -> p t", p=P))

            # ---------------- Phase 0: key/val derivation ----------------
            with tc.tile_pool(name="p0", bufs=1) as p0, \
                 tc.tile_pool(name="p0st", bufs=4) as p0st, \
                 tc.tile_pool(name="ps_vs", bufs=2, space="PSUM") as ps_vs:
                memT = [p0.tile([P, J], bf16, tag=f"memT{m}", name=f"memT{m}")
                        for m in range(MT)]
                fkT = [p0.tile([P, D], bf16, tag=f"fkT{m}", name=f"fkT{m}")
                       for m in range(MT)]
                fvT = [p0.tile([P, D], bf16, tag=f"fvT{m}", name=f"fvT{m}")
                       for m in range(MT)]
                fvb16 = p0.tile([1, D], bf16, tag="fvb16")

                fvb32 = p0st.tile([1, D], f32, tag="fvb32", bufs=1)
                nc.sync.dma_start(out=fvb32,
                                  in_=fvb_d.rearrange("(a d) -> a d", a=1))
                nc.vector.tensor_copy(out=fvb16, in_=fvb32)

                # transpose mem -> memT (cast to bf16 first: bf16 PE
                # transposes are ~2x cheaper than fp32 LOW/HIGH pairs)
                for jt in range(JT):
                    st = p0st.tile([P, MD], f32, tag="st")
                    nc.sync.dma_start(out=st, in_=mem_d[jt * P:(jt + 1) * P, :])
                    st16 = p0st.tile([P, MD], bf16, tag="st16", bufs=2)
                    nc.vector.tensor_copy(out=st16, in_=st)
                    for m in range(MT):
                        pt = ps_t.tile([P, P], bf16, tag="t")
                        nc.tensor.transpose(pt, st16[:, m * P:(m + 1) * P],
                                            ident16)
                        nc.vector.tensor_copy(
                            out=memT[m][:, jt * P:(jt + 1) * P], in_=pt)
                # transpose fk_w, fv_w
                for w_ap, wT in ((fkw_d, fkT), (fvw_d, fvT)):
                    for dt in range(DT):
                        st = p0st.tile([P, MD], f32, tag="st")
                        nc.sync.dma_start(out=st,
                                          in_=w_ap[dt * P:(dt + 1) * P, :])
                        st16 = p0st.tile([P, MD], bf16, tag="st16", bufs=2)
                        nc.vector.tensor_copy(out=st16, in_=st)
                        for m in range(MT):
                            pt = ps_t.tile([P, P], bf16, tag="t")
                            nc.tensor.transpose(pt, st16[:, m * P:(m + 1) * P],
                                                ident16)
                            nc.vector.tensor_copy(
                                out=wT[m][:, dt * P:(dt + 1) * P], in_=pt)

                # ekT8[d-tile] = exp(sum_m fkT[m].T @ memT[m] + fk_b), fp8
                for dt in range(DT):
                    for jc in range(J // 512):
                        ps = ps_s.tile([P, 512], f32, tag="s")
                        for m in range(MT):
                            nc.tensor.matmul(
                                ps,
                                lhsT=fkT[m][:, dt * P:(dt + 1) * P],
                                rhs=memT[m][:, jc * 512:(jc + 1) * 512],
                                start=(m == 0), stop=(m == MT - 1))
                        nc.scalar.activation(
                            out=ekT8[:, dt // 2, dt % 2,
                                     jc * 512:(jc + 1) * 512],
                            in_=ps, func=AF.Exp, bias=fkbT[:, dt:dt + 1],
                            scale=1.0)

                # val8[j-tile] = relu(sum_m memT[m].T @ fvT[m] + 1 x fv_b)
                # and valsum[dd] = colsum(val) accumulated in fp32 psum
                pv0 = ps_vs.tile([1, 512], f32, tag="vs")
                pv1 = ps_vs.tile([1, 512], f32, tag="vs")
                for jt in range(JT):
                    vt16 = p0st.tile([P, D], bf16, tag="vt16", bufs=2)
                    for dh in range(D // 512):
                        ps = ps_s.tile([P, 512], f32, tag="s")
                        for m in range(MT):
                            nc.tensor.matmul(
                                ps,
                                lhsT=memT[m][:, jt * P:(jt + 1) * P],
                                rhs=fvT[m][:, dh * 512:(dh + 1) * 512],
                                start=(m == 0), stop=False)
                        nc.tensor.matmul(
                            ps, lhsT=ones_r16,
                            rhs=fvb16[:, dh * 512:(dh + 1) * 512],
                            start=False, stop=True)
                        nc.scalar.activation(
                            out=vt16[:, dh * 512:(dh + 1) * 512], in_=ps,
                            func=AF.Relu)
                        nc.vector.tensor_scalar_max(
                            val8[:, jt // 2, jt % 2, dh * 512:(dh + 1) * 512],
                            ps, 0.0)
                    nc.tensor.matmul(pv0, lhsT=ones_c16, rhs=vt16[:, 0:512],
                                     start=(jt == 0), stop=(jt == JT - 1))
                    nc.tensor.matmul(pv1, lhsT=ones_c16, rhs=vt16[:, 512:1024],
                                     start=(jt == 0), stop=(jt == JT - 1))
                # broadcast valsum across partitions (rank-1 fp32 matmul)
                vs_row = p0.tile([1, D], f32, tag="vs_row")
                nc.vector.tensor_copy(out=vs_row[:, 0:512], in_=pv0)
                nc.vector.tensor_copy(out=vs_row[:, 512:1024], in_=pv1)
                for dh in range(D // 512):
                    pb = ps_s.tile([P, 512], f32, tag="s")
                    nc.tensor.matmul(pb, lhsT=ones_r32,
                                     rhs=vs_row[:, dh * 512:(dh + 1) * 512],
                                     start=True, stop=True)
                    nc.vector.tensor_copy(
                        out=vsum_bc[:, dh * 512:(dh + 1) * 512], in_=pb)

                # key softmax denominators: c_col[p, jt] = 1/sum_d ekT[:, j]
                for jc in range(J // 512):
                    pd = ps_t.tile([1, 512], f32, tag="t")
                    for dt in range(DT):
                        nc.tensor.matmul(
                            pd, lhsT=ones_c8,
                            rhs=ekT8[:, dt // 2, dt % 2,
                                     jc * 512:(jc + 1) * 512],
                            start=(dt == 0), stop=(dt == DT - 1))
                    crow = p0st.tile([1, 512], f32, tag="crow", bufs=2)
                    nc.vector.tensor_copy(out=crow, in_=pd)
                    pq = ps_t.tile([P, 4], f32, tag="t")
                    for q in range(4):
                        nc.tensor.transpose(pq[:, q:q + 1],
                                            crow[:, q * P:(q + 1) * P],
                                            ident[0:1, 0:1])
                    nc.vector.reciprocal(
                        out=c_col[:, jc * 4:(jc + 1) * 4], in_=pq)

            # ---------------- Phase 1: attention over k rows ----------------
            with tc.tile_pool(name="p1", bufs=2) as p1, \
                 tc.tile_pool(name="p1e", bufs=2) as p1e, \
                 tc.tile_pool(name="ps_o", bufs=2, space="PSUM") as ps_o:
                for ci in range(NC_):
                    n0 = ci * NCHUNK
                    # load + cast + transpose k rows -> kT8[d-pair, n] fp8
                    kT8 = p1.tile([P, DT // 2, 2, NCHUNK], fp8, tag="kT8")
                    for ns in range(NS):
                        st = p1.tile([P, D], f32, tag="kst")
                        nc.sync.dma_start(
                            out=st, in_=k_d[n0 + ns * P:n0 + (ns + 1) * P, :])
                        st16 = p1.tile([P, D], bf16, tag="kst16")
                        nc.vector.tensor_copy(out=st16, in_=st)
                        for dc2 in range(DT // 2):
                            pt = ps_t.tile([P, 2, P], bf16, tag="t")
                            for o in range(2):
                                dc = dc2 * 2 + o
                                nc.tensor.transpose(
                                    pt[:, o, :], st16[:, dc * P:(dc + 1) * P],
                                    ident16)
                            nc.vector.tensor_copy(
                                out=kT8[:, dc2, :, ns * P:(ns + 1) * P],
                                in_=pt)

                    # scoresT (DoubleRow) -> exp in place -> Es = E-1 (fp8)
                    Es8 = p1e.tile([P, JT // 2, 2, NCHUNK], fp8, tag="Es8")
                    for jt in range(JT):
                        ps = ps_s.tile([P, NCHUNK], f32, tag="s")
                        for dc2 in range(DT // 2):
                            nc.tensor.matmul(
                                ps,
                                lhsT=ekT8[:, dc2, :, jt * P:(jt + 1) * P],
                                rhs=kT8[:, dc2, :, :],
                                start=(dc2 == 0), stop=(dc2 == DT // 2 - 1),
                                perf_mode=DR)
                        e16 = p1.tile([P, NCHUNK], bf16, tag="e16", bufs=3)
                        nc.scalar.activation(
                            out=e16, in_=ps, func=AF.Exp,
                            scale=c_col[:, jt:jt + 1])
                        nc.vector.tensor_scalar_add(
                            Es8[:, jt // 2, jt % 2, :], e16, -1.0)

                    # out[n-tile, dd] = (vsum + Es.T @ val) / (J + Es.T @ 1)
                    for ns in range(NS):
                        po0 = ps_o.tile([P, 512], f32, tag="o")
                        po1 = ps_o.tile([P, 512], f32, tag="o")
                        pden = ps_d.tile([P, 1], f32, tag="den")
                        for jc2 in range(JT // 2):
                            lhs = Es8[:, jc2, :, ns * P:(ns + 1) * P]
                            st_, sp_ = (jc2 == 0), (jc2 == JT // 2 - 1)
                            nc.tensor.matmul(po0, lhsT=lhs,
                                             rhs=val8[:, jc2, :, 0:512],
                                             start=st_, stop=sp_, perf_mode=DR)
                            nc.tensor.matmul(po1, lhsT=lhs,
                                             rhs=val8[:, jc2, :, 512:1024],
                                             start=st_, stop=sp_, perf_mode=DR)
                            nc.tensor.matmul(pden, lhsT=lhs,
                                             rhs=ones8[:, :, 0:1],
                                             start=st_, stop=sp_, perf_mode=DR)
                        rv = p1.tile([P, 1], f32, tag="rv")
                        nc.vector.tensor_scalar_add(rv, pden, float(J))
                        nc.vector.reciprocal(out=rv, in_=rv)
                        for dh, po in ((0, po0), (1, po1)):
                            osb = p1.tile([P, 512], f32, tag="osb")
                            nc.vector.tensor_add(
                                osb, po, vsum_bc[:, dh * 512:(dh + 1) * 512])
                            nc.vector.tensor_scalar_mul(osb, osb, rv)
                            nc.sync.dma_start(
                                out=out_d[n0 + ns * P:n0 + (ns + 1) * P,
                                          dh * 512:(dh + 1) * 512],
                                in_=osb)

    nc.compile()
    return nc


def _get_nc():
    if "nc" not in _CACHE:
        _CACHE["nc"] = _build()
    return _CACHE["nc"]


def kernel(**inputs) -> np.ndarray:
    from concourse.bass_utils import run_bass_kernel_spmd

    k = np.ascontiguousarray(np.asarray(inputs["k"], dtype=np.float32))
    mem = np.ascontiguousarray(np.asarray(inputs["mem"], dtype=np.float32))
    fk_w = np.ascontiguousarray(np.asarray(inputs["fk_w"], dtype=np.float32))
    fk_b = np.ascontiguousarray(np.asarray(inputs["fk_b"], dtype=np.float32))
    fv_w = np.ascontiguousarray(np.asarray(inputs["fv_w"], dtype=np.float32))
    fv_b = np.ascontiguousarray(np.asarray(inputs["fv_b"], dtype=np.float32))
    ident = np.eye(P, dtype=np.float32)

    nc = _get_nc()
    in_maps = []
    for c in range(NCORES):
        in_maps.append({
            "k": k[c * S:(c + 1) * S],
            "mem": mem, "fk_w": fk_w, "fk_b": fk_b,
            "fv_w": fv_w, "fv_b": fv_b, "ident": ident,
        })
    res = run_bass_kernel_spmd(nc, in_maps, core_ids=list(range(NCORES)),
                               **_CACHE.get("run_kwargs", {}))
    _CACHE["last_result"] = res
    return np.concatenate([res.results[c]["out"] for c in range(NCORES)],
                          axis=0)


# revision 18
# speedup vs baseline: 1.7450x; 1.0125x over previous
"""MemoryNet kernel for 8 TRN2 NeuronCores (Bass/Tile).

Reference (single-device):
    key = softmax(mem @ fk_w.T + fk_b, axis=-1)      # [J, D]
    val = relu(mem @ fv_w.T + fv_b)                  # [J, D]
    att = softmax(k @ key.T, axis=-1)                # [N, J]
    out = att @ val                                  # [N, D]
with J=4096 (num_mem), MD=512 (mem_dim), D=1024 (inp_dim), N=32768.

Sharding: data-parallel over rows of k (N) across 8 cores; mem + weights
replicated on every core. Each core computes out rows for its shard; the
host concatenates.

Per-core algorithm. Derivation matmuls in bf16; the two big attention
matmuls run in fp8e4m3 with perf_mode=DoubleRow (2 contraction rows per
PE cell). fp8's ~6% relative steps would destroy att's small softmax
deviations if E=exp(s)~1.0 were quantized directly, so we store
Es = E - 1 (|Es|~0.04, 12x better absolute precision) and reconstruct:
    out = (colsum(val) + Es @ val) / (J + Es @ 1)
which matches full-bf16 accuracy (~6e-4 scale-relative, measured).

  Phase 0 (replicated derivation):
    memT/fk_wT/fv_wT via bf16 PE transposes.
    ekT[d,j]  = exp(fk_wT.T @ memT + fk_b[d])  -> fp8, d-pair-interleaved
    c[j]      = 1 / sum_d ekT   (ones matmul + transposed reciprocal)
    val[j,dd] = relu(memT.T @ fv_wT + fv_b)    -> fp8 (+ bf16 temp for
                valsum), fv_b added via rank-1 matmul
    valsum    = colsum(val)  (fp32 psum), broadcast to [128, D]
  Phase 1 (per chunk of NCHUNK k-rows):
    kT8[d,n]  via fp8 PE transposes of DMA'd k rows
    u[j-tile, n] = ekT8.T @ kT8      (DoubleRow, PSUM accumulate over d)
    exp in-place on PSUM (ACT, per-partition scale c_j), then
    Es = u - 1 -> fp8 SBUF (DVE)
    num[n-tile, dd] = Es.T @ val     (DoubleRow, accumulate over j)
    den[n-tile, 1]  = Es.T @ ones    (same lhsT, free-dim-1 matmul)
    out = (num + valsum) * 1/(J + den)
"""

import numpy as np

P = 128
J = 4096      # num_mem
MD = 512      # mem_dim
D = 1024      # inp_dim
NTOT = 32768  # total k rows
NCORES = 8
S = NTOT // NCORES   # k rows per core
NCHUNK = 512         # k rows processed per phase-1 chunk

_CACHE = {}


def _build():
    import concourse.bass as bass
    import concourse.tile as tile
    from concourse import bacc, mybir

    f32 = mybir.dt.float32
    bf16 = mybir.dt.bfloat16
    fp8 = mybir.dt.float8e4
    DR = mybir.MatmulPerfMode.DoubleRow
    AF = mybir.ActivationFunctionType

    nc = bacc.Bacc("TRN2", target_bir_lowering=False, debug=False,
                   num_devices=NCORES)

    k_d = nc.dram_tensor("k", [S, D], f32, kind="ExternalInput").ap()
    mem_d = nc.dram_tensor("mem", [J, MD], f32, kind="ExternalInput").ap()
    fkw_d = nc.dram_tensor("fk_w", [D, MD], f32, kind="ExternalInput").ap()
    fkb_d = nc.dram_tensor("fk_b", [D], f32, kind="ExternalInput").ap()
    fvw_d = nc.dram_tensor("fv_w", [D, MD], f32, kind="ExternalInput").ap()
    fvb_d = nc.dram_tensor("fv_b", [D], f32, kind="ExternalInput").ap()
    id_d = nc.dram_tensor("ident", [P, P], f32, kind="ExternalInput").ap()
    out_d = nc.dram_tensor("out", [S, D], f32, kind="ExternalOutput").ap()

    JT = J // P        # 32 j-tiles
    DT = D // P        # 8 d-tiles
    MT = MD // P       # 4 m-tiles
    NC_ = S // NCHUNK  # phase-1 chunks
    NS = NCHUNK // P   # n-subtiles per chunk

    with tile.TileContext(nc) as tc:
        from contextlib import ExitStack
        ctx = ExitStack()
        with ctx:
            persist = ctx.enter_context(tc.tile_pool(name="persist", bufs=1))
            ps_s = ctx.enter_context(tc.tile_pool(name="ps_s", bufs=4, space="PSUM"))
            ps_d = ctx.enter_context(tc.tile_pool(name="ps_d", bufs=1, space="PSUM"))
            ps_t = ctx.enter_context(tc.tile_pool(name="ps_t", bufs=1, space="PSUM"))

            # persistent tiles.  fp8 operands for DoubleRow matmuls are laid
            # out pair-interleaved: plane [.., i2, o, ..] holds contraction
            # row 256*i2 + 128*o + p.
            ekT8 = persist.tile([P, DT // 2, 2, J], fp8, tag="ekT8")
            val8 = persist.tile([P, JT // 2, 2, D], fp8, tag="val8")
            vsum_bc = persist.tile([P, D], f32, tag="vsum_bc")
            ident = persist.tile([P, P], f32, tag="ident")
            ident16 = persist.tile([P, P], bf16, tag="ident16")
            ident8 = persist.tile([P, P], fp8, tag="ident8")
            ones_c16 = persist.tile([P, 1], bf16, tag="ones_c")   # colsum lhsT
            ones_c8 = persist.tile([P, 1], fp8, tag="ones_c8")    # fp8 colsum lhsT
            ones8 = persist.tile([P, 2, 16], fp8, tag="ones8")    # DR den rhs
            ones_r16 = persist.tile([1, P], bf16, tag="ones_r")   # rank-1 bias lhsT
            ones_r32 = persist.tile([1, P], f32, tag="ones_r32")  # rank-1 f32 lhsT
            fkbT = persist.tile([P, DT], f32, tag="fkbT")
            c_col = persist.tile([P, JT], f32, tag="c_col")       # 1/keysum per j

            nc.sync.dma_start(out=ident, in_=id_d)
            nc.vector.tensor_copy(out=ident16, in_=ident)
            nc.vector.tensor_copy(out=ident8, in_=ident)
            nc.vector.memset(ones_c16, 1.0)
            nc.vector.memset(ones_c8, 1.0)
            nc.vector.memset(ones8, 1.0)
            nc.vector.memset(ones_r16, 1.0)
            nc.vector.memset(ones_r32, 1.0)
            # fk_b -> per-partition layout: fkbT[p, t] = fk_b[t*128 + p]
            nc.sync.dma_start(out=fkbT, in_=fkb_d.rearrange("(t p) -> p t", p=P))

            # ---------------- Phase 0: key/val derivation ----------------
            with tc.tile_pool(name="p0", bufs=1) as p0, \
                 tc.tile_pool(name="p0st", bufs=4) as p0st, \
                 tc.tile_pool(name="ps_vs", bufs=2, space="PSUM") as ps_vs:
                memT = [p0.tile([P, J], bf16, tag=f"memT{m}", name=f"memT{m}")
                        for m in range(MT)]
                fkT = [p0.tile([P, D], bf16, tag=f"fkT{m}", name=f"fkT{m}")
                       for m in range(MT)]
                fvT = [p0.tile([P, D], bf16, tag=f"fvT{m}", name=f"fvT{m}")
                       for m in range(MT)]
                fvb16 = p0.tile([1, D], bf16, tag="fvb16")

                fvb32 = p0st.tile([1, D], f32, tag="fvb32", bufs=1)
                nc.sync.dma_start(out=fvb32,
                                  in_=fvb_d.rearrange("(a d) -> a d", a=1))
                nc.vector.tensor_copy(out=fvb16, in_=fvb32)

                # transpose mem -> memT (cast to bf16 first: bf16 PE
                # transposes are ~2x cheaper than fp32 LOW/HIGH pairs)
                for jt in range(JT):
                    st = p0st.tile([P, MD], f32, tag="st")
                    nc.sync.dma_start(out=st, in_=mem_d[jt * P:(jt + 1) * P, :])
                    st16 = p0st.tile([P, MD], bf16, tag="st16", bufs=2)
                    nc.vector.tensor_copy(out=st16, in_=st)
                    pt = ps_t.tile([P, MT, P], bf16, tag="t")
                    for m in range(MT):
                        nc.tensor.transpose(pt[:, m, :],
                                            st16[:, m * P:(m + 1) * P],
                                            ident16)
                    # single batched evac on ACT (idle in this region; DVE is
                    # the bottleneck during the transpose phase)
                    for m in range(MT):
                        nc.scalar.copy(out=memT[m][:, jt * P:(jt + 1) * P],
                                       in_=pt[:, m, :])
                # transpose fk_w, fv_w
                for w_ap, wT in ((fkw_d, fkT), (fvw_d, fvT)):
                    for dt in range(DT):
                        st = p0st.tile([P, MD], f32, tag="st")
                        nc.sync.dma_start(out=st,
                                          in_=w_ap[dt * P:(dt + 1) * P, :])
                        st16 = p0st.tile([P, MD], bf16, tag="st16", bufs=2)
                        nc.vector.tensor_copy(out=st16, in_=st)
                        pt = ps_t.tile([P, MT, P], bf16, tag="t")
                        for m in range(MT):
                            nc.tensor.transpose(pt[:, m, :],
                                                st16[:, m * P:(m + 1) * P],
                                                ident16)
                        for m in range(MT):
                            nc.scalar.copy(out=wT[m][:, dt * P:(dt + 1) * P],
                                           in_=pt[:, m, :])

                # ekT8[d-tile] = exp(sum_m fkT[m].T @ memT[m] + fk_b), fp8
                for dt in range(DT):
                    for jc in range(J // 512):
                        ps = ps_s.tile([P, 512], f32, tag="s")
                        for m in range(MT):
                            nc.tensor.matmul(
                                ps,
                                lhsT=fkT[m][:, dt * P:(dt + 1) * P],
                                rhs=memT[m][:, jc * 512:(jc + 1) * 512],
                                start=(m == 0), stop=(m == MT - 1))
                        nc.scalar.activation(
                            out=ekT8[:, dt // 2, dt % 2,
                                     jc * 512:(jc + 1) * 512],
                            in_=ps, func=AF.Exp, bias=fkbT[:, dt:dt + 1],
                            scale=1.0)

                # key softmax denominators: c_col[p, jt] = 1/sum_d ekT[:, j]
                for jc in range(J // 512):
                    pd = ps_t.tile([1, 512], f32, tag="t")
                    for dt in range(DT):
                        nc.tensor.matmul(
                            pd, lhsT=ones_c8,
                            rhs=ekT8[:, dt // 2, dt % 2,
                                     jc * 512:(jc + 1) * 512],
                            start=(dt == 0), stop=(dt == DT - 1))
                    crow = p0st.tile([1, 512], f32, tag="crow", bufs=2)
                    nc.vector.tensor_copy(out=crow, in_=pd)
                    pq = ps_t.tile([P, 4], f32, tag="t")
                    for q in range(4):
                        nc.tensor.transpose(pq[:, q:q + 1],
                                            crow[:, q * P:(q + 1) * P],
                                            ident[0:1, 0:1])
                    nc.vector.reciprocal(
                        out=c_col[:, jc * 4:(jc + 1) * 4], in_=pq)

                # val8[j-tile] = relu(sum_m memT[m].T @ fvT[m] + 1 x fv_b)
                # and valsum[dd] = colsum(val) accumulated in fp32 psum
                pv0 = ps_vs.tile([1, 512], f32, tag="vs")
                pv1 = ps_vs.tile([1, 512], f32, tag="vs")
                for jt in range(JT):
                    vt16 = p0st.tile([P, D], bf16, tag="vt16", bufs=2)
                    for dh in range(D // 512):
                        ps = ps_s.tile([P, 512], f32, tag="s")
                        for m in range(MT):
                            nc.tensor.matmul(
                                ps,
                                lhsT=memT[m][:, jt * P:(jt + 1) * P],
                                rhs=fvT[m][:, dh * 512:(dh + 1) * 512],
                                start=(m == 0), stop=False)
                        nc.tensor.matmul(
                            ps, lhsT=ones_r16,
                            rhs=fvb16[:, dh * 512:(dh + 1) * 512],
                            start=False, stop=True)
                        nc.scalar.activation(
                            out=vt16[:, dh * 512:(dh + 1) * 512], in_=ps,
                            func=AF.Relu)
                        nc.vector.tensor_scalar_max(
                            val8[:, jt // 2, jt % 2, dh * 512:(dh + 1) * 512],
                            ps, 0.0)
                    nc.tensor.matmul(pv0, lhsT=ones_c16, rhs=vt16[:, 0:512],
                                     start=(jt == 0), stop=(jt == JT - 1))
                    nc.tensor.matmul(pv1, lhsT=ones_c16, rhs=vt16[:, 512:1024],
                                     start=(jt == 0), stop=(jt == JT - 1))
                # broadcast valsum across partitions (rank-1 fp32 matmul)
                vs_row = p0.tile([1, D], f32, tag="vs_row")
                nc.vector.tensor_copy(out=vs_row[:, 0:512], in_=pv0)
                nc.vector.tensor_copy(out=vs_row[:, 512:1024], in_=pv1)
                for dh in range(D // 512):
                    pb = ps_s.tile([P, 512], f32, tag="s")
                    nc.tensor.matmul(pb, lhsT=ones_r32,
                                     rhs=vs_row[:, dh * 512:(dh + 1) * 512],
                                     start=True, stop=True)
                    nc.vector.tensor_copy(
                        out=vsum_bc[:, dh * 512:(dh + 1) * 512], in_=pb)

            # ---------------- Phase 1: attention over k rows ----------------
            with tc.tile_pool(name="p1", bufs=2) as p1, \
                 tc.tile_pool(name="p1e", bufs=2) as p1e, \
                 tc.tile_pool(name="ps_o", bufs=2, space="PSUM") as ps_o:
                for ci in range(NC_):
                    n0 = ci * NCHUNK
                    # load + cast + transpose k rows -> kT8[d-pair, n] fp8
                    kT8 = p1.tile([P, DT // 2, 2, NCHUNK], fp8, tag="kT8")
                    for ns in range(NS):
                        st = p1.tile([P, D], f32, tag="kst")
                        nc.sync.dma_start(
                            out=st, in_=k_d[n0 + ns * P:n0 + (ns + 1) * P, :])
                        st16 = p1.tile([P, D], bf16, tag="kst16")
                        nc.vector.tensor_copy(out=st16, in_=st)
                        for dc2 in range(DT // 2):
                            pt = ps_t.tile([P, 2, P], bf16, tag="t")
                            for o in range(2):
                                dc = dc2 * 2 + o
                                nc.tensor.transpose(
                                    pt[:, o, :], st16[:, dc * P:(dc + 1) * P],
                                    ident16)
                            nc.vector.tensor_copy(
                                out=kT8[:, dc2, :, ns * P:(ns + 1) * P],
                                in_=pt)

                    # scoresT (DoubleRow) -> exp in place -> Es = E-1 (fp8)
                    Es8 = p1e.tile([P, JT // 2, 2, NCHUNK], fp8, tag="Es8")
                    for jt in range(JT):
                        ps = ps_s.tile([P, NCHUNK], f32, tag="s")
                        for dc2 in range(DT // 2):
                            nc.tensor.matmul(
                                ps,
                                lhsT=ekT8[:, dc2, :, jt * P:(jt + 1) * P],
                                rhs=kT8[:, dc2, :, :],
                                start=(dc2 == 0), stop=(dc2 == DT // 2 - 1),
                                perf_mode=DR)
                        e16 = p1.tile([P, NCHUNK], bf16, tag="e16", bufs=3)
                        nc.scalar.activation(
                            out=e16, in_=ps, func=AF.Exp,
                            scale=c_col[:, jt:jt + 1])
                        nc.vector.tensor_scalar_add(
                            Es8[:, jt // 2, jt % 2, :], e16, -1.0)

                    # out[n-tile, dd] = (vsum + Es.T @ val) / (J + Es.T @ 1)
                    for ns in range(NS):
                        po0 = ps_o.tile([P, 512], f32, tag="o")
                        po1 = ps_o.tile([P, 512], f32, tag="o")
                        pden = ps_d.tile([P, 1], f32, tag="den")
                        for jc2 in range(JT // 2):
                            lhs = Es8[:, jc2, :, ns * P:(ns + 1) * P]
                            st_, sp_ = (jc2 == 0), (jc2 == JT // 2 - 1)
                            nc.tensor.matmul(po0, lhsT=lhs,
                                             rhs=val8[:, jc2, :, 0:512],
                                             start=st_, stop=sp_, perf_mode=DR)
                            nc.tensor.matmul(po1, lhsT=lhs,
                                             rhs=val8[:, jc2, :, 512:1024],
                                             start=st_, stop=sp_, perf_mode=DR)
                            nc.tensor.matmul(pden, lhsT=lhs,
                                             rhs=ones8[:, :, 0:1],
                                             start=st_, stop=sp_, perf_mode=DR)
                        rv = p1.tile([P, 1], f32, tag="rv")
                        nc.vector.tensor_scalar_add(rv, pden, float(J))
                        nc.vector.reciprocal(out=rv, in_=rv)
                        for dh, po in ((0, po0), (1, po1)):
                            osb = p1.tile([P, 512], f32, tag="osb")
                            nc.vector.tensor_add(
                                osb, po, vsum_bc[:, dh * 512:(dh + 1) * 512])
                            nc.vector.tensor_scalar_mul(osb, osb, rv)
                            nc.sync.dma_start(
                                out=out_d[n0 + ns * P:n0 + (ns + 1) * P,
                                          dh * 512:(dh + 1) * 512],
                                in_=osb)

    nc.compile()
    return nc


def _get_nc():
    if "nc" not in _CACHE:
        _CACHE["nc"] = _build()
    return _CACHE["nc"]


def kernel(**inputs) -> np.ndarray:
    from concourse.bass_utils import run_bass_kernel_spmd

    k = np.ascontiguousarray(np.asarray(inputs["k"], dtype=np.float32))
    mem = np.ascontiguousarray(np.asarray(inputs["mem"], dtype=np.float32))
    fk_w = np.ascontiguousarray(np.asarray(inputs["fk_w"], dtype=np.float32))
    fk_b = np.ascontiguousarray(np.asarray(inputs["fk_b"], dtype=np.float32))
    fv_w = np.ascontiguousarray(np.asarray(inputs["fv_w"], dtype=np.float32))
    fv_b = np.ascontiguousarray(np.asarray(inputs["fv_b"], dtype=np.float32))
    ident = np.eye(P, dtype=np.float32)

    nc = _get_nc()
    in_maps = []
    for c in range(NCORES):
        in_maps.append({
            "k": k[c * S:(c + 1) * S],
            "mem": mem, "fk_w": fk_w, "fk_b": fk_b,
            "fv_w": fv_w, "fv_b": fv_b, "ident": ident,
        })
    res = run_bass_kernel_spmd(nc, in_maps, core_ids=list(range(NCORES)),
                               **_CACHE.get("run_kwargs", {}))
    _CACHE["last_result"] = res
    return np.concatenate([res.results[c]["out"] for c in range(NCORES)],
                          axis=0)


# revision 19
# speedup vs baseline: 1.8783x; 1.0764x over previous
"""MemoryNet kernel for 8 TRN2 NeuronCores (Bass/Tile).

Reference (single-device):
    key = softmax(mem @ fk_w.T + fk_b, axis=-1)      # [J, D]
    val = relu(mem @ fv_w.T + fv_b)                  # [J, D]
    att = softmax(k @ key.T, axis=-1)                # [N, J]
    out = att @ val                                  # [N, D]
with J=4096 (num_mem), MD=512 (mem_dim), D=1024 (inp_dim), N=32768.

Sharding: data-parallel over rows of k (N) across 8 cores; mem + weights
replicated on every core. Each core computes out rows for its shard; the
host concatenates.

Per-core algorithm. Derivation matmuls in bf16; the two big attention
matmuls run in fp8e4m3 with perf_mode=DoubleRow (2 contraction rows per
PE cell). fp8's ~6% relative steps would destroy att's small softmax
deviations if E=exp(s)~1.0 were quantized directly, so we store
Es = E - 1 (|Es|~0.04, 12x better absolute precision) and reconstruct:
    out = (colsum(val) + Es @ val) / (J + Es @ 1)
which matches full-bf16 accuracy (~6e-4 scale-relative, measured).

  Phase 0 (replicated derivation):
    memT/fk_wT/fv_wT via bf16 PE transposes.
    ekT[d,j]  = exp(fk_wT.T @ memT + fk_b[d])  -> fp8, d-pair-interleaved
    c[j]      = 1 / sum_d ekT   (ones matmul + transposed reciprocal)
    val[j,dd] = relu(memT.T @ fv_wT + fv_b)    -> fp8 (+ bf16 temp for
                valsum), fv_b added via rank-1 matmul
    valsum    = colsum(val)  (fp32 psum), broadcast to [128, D]
  Phase 1 (per chunk of NCHUNK k-rows):
    kT8[d,n]  via fp8 PE transposes of DMA'd k rows
    u[j-tile, n] = ekT8.T @ kT8      (DoubleRow, PSUM accumulate over d)
    exp in-place on PSUM (ACT, per-partition scale c_j), then
    Es = u - 1 -> fp8 SBUF (DVE)
    num[n-tile, dd] = Es.T @ val     (DoubleRow, accumulate over j)
    den[n-tile, 1]  = Es.T @ ones    (same lhsT, free-dim-1 matmul)
    out = (num + valsum) * 1/(J + den)
"""

import numpy as np

P = 128
J = 4096      # num_mem
MD = 512      # mem_dim
D = 1024      # inp_dim
NTOT = 32768  # total k rows
NCORES = 8
S = NTOT // NCORES   # k rows per core
NCHUNK = 512         # k rows processed per phase-1 chunk

_CACHE = {}


def _build():
    import concourse.bass as bass
    import concourse.tile as tile
    from concourse import bacc, mybir

    f32 = mybir.dt.float32
    bf16 = mybir.dt.bfloat16
    fp8 = mybir.dt.float8e4
    DR = mybir.MatmulPerfMode.DoubleRow
    AF = mybir.ActivationFunctionType

    nc = bacc.Bacc("TRN2", target_bir_lowering=False, debug=False,
                   num_devices=NCORES)

    k_d = nc.dram_tensor("k", [S, D], f32, kind="ExternalInput").ap()
    mem_d = nc.dram_tensor("mem", [J, MD], f32, kind="ExternalInput").ap()
    fkw_d = nc.dram_tensor("fk_w", [D, MD], f32, kind="ExternalInput").ap()
    fkb_d = nc.dram_tensor("fk_b", [D], f32, kind="ExternalInput").ap()
    fvw_d = nc.dram_tensor("fv_w", [D, MD], f32, kind="ExternalInput").ap()
    fvb_d = nc.dram_tensor("fv_b", [D], f32, kind="ExternalInput").ap()
    id_d = nc.dram_tensor("ident", [P, P], f32, kind="ExternalInput").ap()
    out_d = nc.dram_tensor("out", [S, D], f32, kind="ExternalOutput").ap()

    JT = J // P        # 32 j-tiles
    DT = D // P        # 8 d-tiles
    MT = MD // P       # 4 m-tiles
    NC_ = S // NCHUNK  # phase-1 chunks
    NS = NCHUNK // P   # n-subtiles per chunk

    with tile.TileContext(nc) as tc:
        from contextlib import ExitStack
        ctx = ExitStack()
        with ctx:
            persist = ctx.enter_context(tc.tile_pool(name="persist", bufs=1))
            ps_s = ctx.enter_context(tc.tile_pool(name="ps_s", bufs=3, space="PSUM"))
            ps_d = ctx.enter_context(tc.tile_pool(name="ps_d", bufs=1, space="PSUM"))
            ps_t = ctx.enter_context(tc.tile_pool(name="ps_t", bufs=2, space="PSUM"))

            # persistent tiles.  fp8 operands for DoubleRow matmuls are laid
            # out pair-interleaved: plane [.., i2, o, ..] holds contraction
            # row 256*i2 + 128*o + p.
            ekT8 = persist.tile([P, DT // 2, 2, J], fp8, tag="ekT8")
            val8 = persist.tile([P, JT // 2, 2, D], fp8, tag="val8")
            vsum_bc = persist.tile([P, D], f32, tag="vsum_bc")
            ident = persist.tile([P, P], f32, tag="ident")
            ident16 = persist.tile([P, P], bf16, tag="ident16")
            ident8 = persist.tile([P, P], fp8, tag="ident8")
            ones_c16 = persist.tile([P, 1], bf16, tag="ones_c")   # colsum lhsT
            ones_c8 = persist.tile([P, 1], fp8, tag="ones_c8")    # fp8 colsum lhsT
            ones8 = persist.tile([P, 2, 16], fp8, tag="ones8")    # DR den rhs
            ones_r16 = persist.tile([1, P], bf16, tag="ones_r")   # rank-1 bias lhsT
            ones_r32 = persist.tile([1, P], f32, tag="ones_r32")  # rank-1 f32 lhsT
            fkbT = persist.tile([P, DT], f32, tag="fkbT")
            c_col = persist.tile([P, JT], f32, tag="c_col")       # 1/keysum per j

            nc.sync.dma_start(out=ident, in_=id_d)
            nc.vector.tensor_copy(out=ident16, in_=ident)
            nc.vector.tensor_copy(out=ident8, in_=ident)
            nc.vector.memset(ones_c16, 1.0)
            nc.vector.memset(ones_c8, 1.0)
            nc.vector.memset(ones8, 1.0)
            nc.vector.memset(ones_r16, 1.0)
            nc.vector.memset(ones_r32, 1.0)
            # fk_b -> per-partition layout: fkbT[p, t] = fk_b[t*128 + p]
            nc.sync.dma_start(out=fkbT, in_=fkb_d.rearrange("(t p) -> p t", p=P))

            # k-chunk load/cast/transpose chain.  Defined up front so
            # chunk 0 can be prefetched during phase 0 (it has no dependency
            # on the derivation).
            p1k = ctx.enter_context(tc.tile_pool(name="p1k", bufs=2))

            def load_kT(ci):
                n0 = ci * NCHUNK
                kT8 = p1k.tile([P, DT // 2, 2, NCHUNK], fp8, tag="kT8",
                               name=f"kT8_{ci}")
                for ns in range(NS):
                    st = p1k.tile([P, D], f32, tag="kst", bufs=4,
                                  name=f"kst_{ci}_{ns}")
                    nc.sync.dma_start(
                        out=st, in_=k_d[n0 + ns * P:n0 + (ns + 1) * P, :])
                    st16 = p1k.tile([P, D], bf16, tag="kst16",
                                    name=f"kst16_{ci}_{ns}")
                    nc.vector.tensor_copy(out=st16, in_=st)
                    for dc2 in range(DT // 2):
                        pt = ps_t.tile([P, 2, P], bf16, tag="t",
                                       name=f"ptk_{ci}_{ns}_{dc2}")
                        for o in range(2):
                            dc = dc2 * 2 + o
                            nc.tensor.transpose(
                                pt[:, o, :], st16[:, dc * P:(dc + 1) * P],
                                ident16)
                        nc.vector.tensor_copy(
                            out=kT8[:, dc2, :, ns * P:(ns + 1) * P], in_=pt)
                return kT8

            # ---------------- Phase 0: key/val derivation ----------------
            # Single interleaved stream: weight transposes, then per mem
            # j-tile: transpose -> val group; every 4th tile also the ekT
            # groups and key-denominator chain for that 512-wide j-chunk.
            with tc.tile_pool(name="p0", bufs=1) as p0, \
                 tc.tile_pool(name="p0st", bufs=4) as p0st, \
                 tc.tile_pool(name="ps_vs", bufs=2, space="PSUM") as ps_vs:
                memT = [p0.tile([P, J], bf16, tag=f"memT{m}", name=f"memT{m}")
                        for m in range(MT)]
                fkT = [p0.tile([P, D], bf16, tag=f"fkT{m}", name=f"fkT{m}")
                       for m in range(MT)]
                fvT = [p0.tile([P, D], bf16, tag=f"fvT{m}", name=f"fvT{m}")
                       for m in range(MT)]
                fvb16 = p0.tile([1, D], bf16, tag="fvb16")

                fvb32 = p0st.tile([1, D], f32, tag="fvb32", bufs=1)
                nc.sync.dma_start(out=fvb32,
                                  in_=fvb_d.rearrange("(a d) -> a d", a=1))
                nc.vector.tensor_copy(out=fvb16, in_=fvb32)

                # transpose fk_w, fv_w first (ekT groups need all of fkT)
                for w_ap, wT in ((fkw_d, fkT), (fvw_d, fvT)):
                    for dt in range(DT):
                        st = p0st.tile([P, MD], f32, tag="st")
                        nc.sync.dma_start(out=st,
                                          in_=w_ap[dt * P:(dt + 1) * P, :])
                        st16 = p0st.tile([P, MD], bf16, tag="st16", bufs=2)
                        nc.vector.tensor_copy(out=st16, in_=st)
                        pt = ps_t.tile([P, MT, P], bf16, tag="t")
                        for m in range(MT):
                            nc.tensor.transpose(pt[:, m, :],
                                                st16[:, m * P:(m + 1) * P],
                                                ident16)
                        for m in range(MT):
                            nc.scalar.copy(out=wT[m][:, dt * P:(dt + 1) * P],
                                           in_=pt[:, m, :])

                # prefetch chunk 0's kT (independent of the derivation)
                kT8_pre = load_kT(0)

                pv0 = ps_vs.tile([1, 512], f32, tag="vs")
                pv1 = ps_vs.tile([1, 512], f32, tag="vs")
                for jt in range(JT):
                    # mem[jt] -> memT (batched ACT evac; DVE is busy casting)
                    st = p0st.tile([P, MD], f32, tag="st")
                    nc.sync.dma_start(out=st, in_=mem_d[jt * P:(jt + 1) * P, :])
                    st16 = p0st.tile([P, MD], bf16, tag="st16", bufs=2)
                    nc.vector.tensor_copy(out=st16, in_=st)
                    pt = ps_t.tile([P, MT, P], bf16, tag="t")
                    for m in range(MT):
                        nc.tensor.transpose(pt[:, m, :],
                                            st16[:, m * P:(m + 1) * P],
                                            ident16)
                    for m in range(MT):
                        nc.scalar.copy(out=memT[m][:, jt * P:(jt + 1) * P],
                                       in_=pt[:, m, :])

                    # val8[jt] = relu(sum_m memT[m].T @ fvT[m] + 1 x fv_b)
                    vt16 = p0st.tile([P, D], bf16, tag="vt16", bufs=2)
                    for dh in range(D // 512):
                        ps = ps_s.tile([P, 512], f32, tag="s")
                        for m in range(MT):
                            nc.tensor.matmul(
                                ps,
                                lhsT=memT[m][:, jt * P:(jt + 1) * P],
                                rhs=fvT[m][:, dh * 512:(dh + 1) * 512],
                                start=(m == 0), stop=False)
                        nc.tensor.matmul(
                            ps, lhsT=ones_r16,
                            rhs=fvb16[:, dh * 512:(dh + 1) * 512],
                            start=False, stop=True)
                        nc.scalar.activation(
                            out=vt16[:, dh * 512:(dh + 1) * 512], in_=ps,
                            func=AF.Relu)
                        nc.vector.tensor_scalar_max(
                            val8[:, jt // 2, jt % 2, dh * 512:(dh + 1) * 512],
                            ps, 0.0)
                    nc.tensor.matmul(pv0, lhsT=ones_c16, rhs=vt16[:, 0:512],
                                     start=(jt == 0), stop=(jt == JT - 1))
                    nc.tensor.matmul(pv1, lhsT=ones_c16, rhs=vt16[:, 512:1024],
                                     start=(jt == 0), stop=(jt == JT - 1))

                    if jt % 4 != 3:
                        continue
                    jc = jt // 4
                    # ekT8 groups for this 512-wide j-chunk
                    for dt in range(DT):
                        ps = ps_s.tile([P, 512], f32, tag="s")
                        for m in range(MT):
                            nc.tensor.matmul(
                                ps,
                                lhsT=fkT[m][:, dt * P:(dt + 1) * P],
                                rhs=memT[m][:, jc * 512:(jc + 1) * 512],
                                start=(m == 0), stop=(m == MT - 1))
                        nc.scalar.activation(
                            out=ekT8[:, dt // 2, dt % 2,
                                     jc * 512:(jc + 1) * 512],
                            in_=ps, func=AF.Exp, bias=fkbT[:, dt:dt + 1],
                            scale=1.0)
                    # key softmax denominators -> c_col[:, jc*4:(jc+1)*4]
                    pd = ps_d.tile([1, 512], f32, tag="den")
                    for dt in range(DT):
                        nc.tensor.matmul(
                            pd, lhsT=ones_c8,
                            rhs=ekT8[:, dt // 2, dt % 2,
                                     jc * 512:(jc + 1) * 512],
                            start=(dt == 0), stop=(dt == DT - 1))
                    crow = p0st.tile([1, 512], f32, tag="crow", bufs=2)
                    nc.vector.tensor_copy(out=crow, in_=pd)
                    pq = ps_d.tile([P, 4], f32, tag="den")
                    for q in range(4):
                        nc.tensor.transpose(pq[:, q:q + 1],
                                            crow[:, q * P:(q + 1) * P],
                                            ident[0:1, 0:1])
                    nc.vector.reciprocal(
                        out=c_col[:, jc * 4:(jc + 1) * 4], in_=pq)

                # broadcast valsum across partitions (rank-1 fp32 matmul)
                vs_row = p0.tile([1, D], f32, tag="vs_row")
                nc.vector.tensor_copy(out=vs_row[:, 0:512], in_=pv0)
                nc.vector.tensor_copy(out=vs_row[:, 512:1024], in_=pv1)
                for dh in range(D // 512):
                    pb = ps_s.tile([P, 512], f32, tag="s")
                    nc.tensor.matmul(pb, lhsT=ones_r32,
                                     rhs=vs_row[:, dh * 512:(dh + 1) * 512],
                                     start=True, stop=True)
                    nc.vector.tensor_copy(
                        out=vsum_bc[:, dh * 512:(dh + 1) * 512], in_=pb)

            # ---------------- Phase 1: attention over k rows ----------------
            with tc.tile_pool(name="p1", bufs=2) as p1, \
                 tc.tile_pool(name="p1e", bufs=2) as p1e, \
                 tc.tile_pool(name="ps_o", bufs=2, space="PSUM") as ps_o:
                for ci in range(NC_):
                    n0 = ci * NCHUNK
                    kT8 = kT8_pre if ci == 0 else load_kT(ci)

                    # scoresT (DoubleRow) -> exp -> Es = E-1 (fp8)
                    Es8 = p1e.tile([P, JT // 2, 2, NCHUNK], fp8, tag="Es8")
                    for jt in range(JT):
                        ps = ps_s.tile([P, NCHUNK], f32, tag="s")
                        for dc2 in range(DT // 2):
                            nc.tensor.matmul(
                                ps,
                                lhsT=ekT8[:, dc2, :, jt * P:(jt + 1) * P],
                                rhs=kT8[:, dc2, :, :],
                                start=(dc2 == 0), stop=(dc2 == DT // 2 - 1),
                                perf_mode=DR)
                        e16 = p1.tile([P, NCHUNK], bf16, tag="e16", bufs=3)
                        nc.scalar.activation(
                            out=e16, in_=ps, func=AF.Exp,
                            scale=c_col[:, jt:jt + 1])
                        nc.vector.tensor_scalar_add(
                            Es8[:, jt // 2, jt % 2, :], e16, -1.0)

                    # out[n-tile, dd] = (vsum + Es.T @ val) / (J + Es.T @ 1)
                    for ns in range(NS):
                        po0 = ps_o.tile([P, 512], f32, tag="o")
                        po1 = ps_o.tile([P, 512], f32, tag="o")
                        pden = ps_d.tile([P, 1], f32, tag="den")
                        for jc2 in range(JT // 2):
                            lhs = Es8[:, jc2, :, ns * P:(ns + 1) * P]
                            st_, sp_ = (jc2 == 0), (jc2 == JT // 2 - 1)
                            nc.tensor.matmul(po0, lhsT=lhs,
                                             rhs=val8[:, jc2, :, 0:512],
                                             start=st_, stop=sp_, perf_mode=DR)
                            nc.tensor.matmul(po1, lhsT=lhs,
                                             rhs=val8[:, jc2, :, 512:1024],
                                             start=st_, stop=sp_, perf_mode=DR)
                            nc.tensor.matmul(pden, lhsT=lhs,
                                             rhs=ones8[:, :, 0:1],
                                             start=st_, stop=sp_, perf_mode=DR)
                        rv = p1.tile([P, 1], f32, tag="rv")
                        nc.vector.tensor_scalar_add(rv, pden, float(J))
                        nc.vector.reciprocal(out=rv, in_=rv)
                        for dh, po in ((0, po0), (1, po1)):
                            osb = p1.tile([P, 512], f32, tag="osb")
                            nc.vector.tensor_add(
                                osb, po, vsum_bc[:, dh * 512:(dh + 1) * 512])
                            nc.vector.tensor_scalar_mul(osb, osb, rv)
                            nc.sync.dma_start(
                                out=out_d[n0 + ns * P:n0 + (ns + 1) * P,
                                          dh * 512:(dh + 1) * 512],
                                in_=osb)

    nc.compile()
    return nc


def _get_nc():
    if "nc" not in _CACHE:
        _CACHE["nc"] = _build()
    return _CACHE["nc"]


def kernel(**inputs) -> np.ndarray:
    from concourse.bass_utils import run_bass_kernel_spmd

    k = np.ascontiguousarray(np.asarray(inputs["k"], dtype=np.float32))
    mem = np.ascontiguousarray(np.asarray(inputs["mem"], dtype=np.float32))
    fk_w = np.ascontiguousarray(np.asarray(inputs["fk_w"], dtype=np.float32))
    fk_b = np.ascontiguousarray(np.asarray(inputs["fk_b"], dtype=np.float32))
    fv_w = np.ascontiguousarray(np.asarray(inputs["fv_w"], dtype=np.float32))
    fv_b = np.ascontiguousarray(np.asarray(inputs["fv_b"], dtype=np.float32))
    ident = np.eye(P, dtype=np.float32)

    nc = _get_nc()
    in_maps = []
    for c in range(NCORES):
        in_maps.append({
            "k": k[c * S:(c + 1) * S],
            "mem": mem, "fk_w": fk_w, "fk_b": fk_b,
            "fv_w": fv_w, "fv_b": fv_b, "ident": ident,
        })
    res = run_bass_kernel_spmd(nc, in_maps, core_ids=list(range(NCORES)),
                               **_CACHE.get("run_kwargs", {}))
    _CACHE["last_result"] = res
    return np.concatenate([res.results[c]["out"] for c in range(NCORES)],
                          axis=0)


# revision 20
# speedup vs baseline: 1.9052x; 1.0143x over previous
"""MemoryNet kernel for 8 TRN2 NeuronCores (Bass/Tile).

Reference (single-device):
    key = softmax(mem @ fk_w.T + fk_b, axis=-1)      # [J, D]
    val = relu(mem @ fv_w.T + fv_b)                  # [J, D]
    att = softmax(k @ key.T, axis=-1)                # [N, J]
    out = att @ val                                  # [N, D]
with J=4096 (num_mem), MD=512 (mem_dim), D=1024 (inp_dim), N=32768.

Sharding: data-parallel over rows of k (N) across 8 cores; mem + weights
replicated on every core. Each core computes out rows for its shard; the
host concatenates.

Per-core algorithm. Derivation matmuls in bf16; the two big attention
matmuls run in fp8e4m3 with perf_mode=DoubleRow (2 contraction rows per
PE cell). fp8's ~6% relative steps would destroy att's small softmax
deviations if E=exp(s)~1.0 were quantized directly, so we store
Es = E - 1 (|Es|~0.04, 12x better absolute precision) and reconstruct:
    out = (colsum(val) + Es @ val) / (J + Es @ 1)
which matches full-bf16 accuracy (~6e-4 scale-relative, measured).

  Phase 0 (replicated derivation):
    memT/fk_wT/fv_wT via bf16 PE transposes.
    ekT[d,j]  = exp(fk_wT.T @ memT + fk_b[d])  -> fp8, d-pair-interleaved
    c[j]      = 1 / sum_d ekT   (ones matmul + transposed reciprocal)
    val[j,dd] = relu(memT.T @ fv_wT + fv_b)    -> fp8 (+ bf16 temp for
                valsum), fv_b added via rank-1 matmul
    valsum    = colsum(val)  (fp32 psum), broadcast to [128, D]
  Phase 1 (per chunk of NCHUNK k-rows):
    kT8[d,n]  via fp8 PE transposes of DMA'd k rows
    u[j-tile, n] = ekT8.T @ kT8      (DoubleRow, PSUM accumulate over d)
    exp in-place on PSUM (ACT, per-partition scale c_j), then
    Es = u - 1 -> fp8 SBUF (DVE)
    num[n-tile, dd] = Es.T @ val     (DoubleRow, accumulate over j)
    den[n-tile, 1]  = Es.T @ ones    (same lhsT, free-dim-1 matmul)
    out = (num + valsum) * 1/(J + den)
"""

import numpy as np

P = 128
J = 4096      # num_mem
MD = 512      # mem_dim
D = 1024      # inp_dim
NTOT = 32768  # total k rows
NCORES = 8
S = NTOT // NCORES   # k rows per core
NCHUNK = 512         # k rows processed per phase-1 chunk

_CACHE = {}


def _build():
    import concourse.bass as bass
    import concourse.tile as tile
    from concourse import bacc, mybir

    f32 = mybir.dt.float32
    bf16 = mybir.dt.bfloat16
    fp8 = mybir.dt.float8e4
    DR = mybir.MatmulPerfMode.DoubleRow
    AF = mybir.ActivationFunctionType

    nc = bacc.Bacc("TRN2", target_bir_lowering=False, debug=False,
                   num_devices=NCORES)

    k_d = nc.dram_tensor("k", [S, D], f32, kind="ExternalInput").ap()
    mem_d = nc.dram_tensor("mem", [J, MD], f32, kind="ExternalInput").ap()
    fkw_d = nc.dram_tensor("fk_w", [D, MD], f32, kind="ExternalInput").ap()
    fkb_d = nc.dram_tensor("fk_b", [D], f32, kind="ExternalInput").ap()
    fvw_d = nc.dram_tensor("fv_w", [D, MD], f32, kind="ExternalInput").ap()
    fvb_d = nc.dram_tensor("fv_b", [D], f32, kind="ExternalInput").ap()
    id_d = nc.dram_tensor("ident", [P, P], f32, kind="ExternalInput").ap()
    out_d = nc.dram_tensor("out", [S, D], f32, kind="ExternalOutput").ap()

    JT = J // P        # 32 j-tiles
    DT = D // P        # 8 d-tiles
    MT = MD // P       # 4 m-tiles
    NC_ = S // NCHUNK  # phase-1 chunks
    NS = NCHUNK // P   # n-subtiles per chunk

    with tile.TileContext(nc) as tc:
        from contextlib import ExitStack
        ctx = ExitStack()
        with ctx:
            persist = ctx.enter_context(tc.tile_pool(name="persist", bufs=1))
            ps_s = ctx.enter_context(tc.tile_pool(name="ps_s", bufs=3, space="PSUM"))
            ps_d = ctx.enter_context(tc.tile_pool(name="ps_d", bufs=1, space="PSUM"))
            ps_t = ctx.enter_context(tc.tile_pool(name="ps_t", bufs=2, space="PSUM"))

            # persistent tiles.  fp8 operands for DoubleRow matmuls are laid
            # out pair-interleaved: plane [.., i2, o, ..] holds contraction
            # row 256*i2 + 128*o + p.
            ekT8 = persist.tile([P, DT // 2, 2, J], fp8, tag="ekT8")
            val8 = persist.tile([P, JT // 2, 2, D], fp8, tag="val8")
            vsum_bc = persist.tile([P, D], f32, tag="vsum_bc")
            ident = persist.tile([P, P], f32, tag="ident")
            ident16 = persist.tile([P, P], bf16, tag="ident16")
            ident8 = persist.tile([P, P], fp8, tag="ident8")
            ones_c16 = persist.tile([P, 1], bf16, tag="ones_c")   # colsum lhsT
            ones_c8 = persist.tile([P, 1], fp8, tag="ones_c8")    # fp8 colsum lhsT
            ones8 = persist.tile([P, 2, 16], fp8, tag="ones8")    # DR den rhs
            ones_r16 = persist.tile([1, P], bf16, tag="ones_r")   # rank-1 bias lhsT
            ones_r32 = persist.tile([1, P], f32, tag="ones_r32")  # rank-1 f32 lhsT
            fkbT = persist.tile([P, DT], f32, tag="fkbT")
            c_col = persist.tile([P, JT], f32, tag="c_col")       # 1/keysum per j

            nc.sync.dma_start(out=ident, in_=id_d)
            nc.vector.tensor_copy(out=ident16, in_=ident)
            nc.vector.tensor_copy(out=ident8, in_=ident)
            nc.vector.memset(ones_c16, 1.0)
            nc.vector.memset(ones_c8, 1.0)
            nc.vector.memset(ones8, 1.0)
            nc.vector.memset(ones_r16, 1.0)
            nc.vector.memset(ones_r32, 1.0)
            # fk_b -> per-partition layout: fkbT[p, t] = fk_b[t*128 + p]
            nc.sync.dma_start(out=fkbT, in_=fkb_d.rearrange("(t p) -> p t", p=P))

            # k-chunk load/cast/transpose chain.  Defined up front so
            # chunk 0 can be prefetched during phase 0 (it has no dependency
            # on the derivation).
            p1k = ctx.enter_context(tc.tile_pool(name="p1k", bufs=2))

            def load_kT(ci):
                n0 = ci * NCHUNK
                kT8 = p1k.tile([P, DT // 2, 2, NCHUNK], fp8, tag="kT8",
                               name=f"kT8_{ci}")
                for ns in range(NS):
                    st = p1k.tile([P, D], f32, tag="kst", bufs=4,
                                  name=f"kst_{ci}_{ns}")
                    nc.sync.dma_start(
                        out=st, in_=k_d[n0 + ns * P:n0 + (ns + 1) * P, :])
                    st16 = p1k.tile([P, D], bf16, tag="kst16",
                                    name=f"kst16_{ci}_{ns}")
                    nc.vector.tensor_copy(out=st16, in_=st)
                    for dc2 in range(DT // 2):
                        pt = ps_t.tile([P, 2, P], bf16, tag="t",
                                       name=f"ptk_{ci}_{ns}_{dc2}")
                        for o in range(2):
                            dc = dc2 * 2 + o
                            nc.tensor.transpose(
                                pt[:, o, :], st16[:, dc * P:(dc + 1) * P],
                                ident16)
                        nc.vector.tensor_copy(
                            out=kT8[:, dc2, :, ns * P:(ns + 1) * P], in_=pt)
                return kT8

            # ---------------- Phase 0: key/val derivation ----------------
            # Single interleaved stream: weight transposes, then per mem
            # j-tile: transpose -> val group; every 4th tile also the ekT
            # groups and key-denominator chain for that 512-wide j-chunk.
            with tc.tile_pool(name="p0", bufs=1) as p0, \
                 tc.tile_pool(name="p0st", bufs=4) as p0st, \
                 tc.tile_pool(name="ps_vs", bufs=2, space="PSUM") as ps_vs:
                memT = [p0.tile([P, J], bf16, tag=f"memT{m}", name=f"memT{m}")
                        for m in range(MT)]
                memT8 = p0.tile([P, MT // 2, 2, J], fp8, tag="memT8")
                fkT8 = p0.tile([P, MT // 2, 2, D], fp8, tag="fkT8")
                fvT = [p0.tile([P, D], bf16, tag=f"fvT{m}", name=f"fvT{m}")
                       for m in range(MT)]
                fvb16 = p0.tile([1, D], bf16, tag="fvb16")

                fvb32 = p0st.tile([1, D], f32, tag="fvb32", bufs=1)
                nc.sync.dma_start(out=fvb32,
                                  in_=fvb_d.rearrange("(a d) -> a d", a=1))
                nc.vector.tensor_copy(out=fvb16, in_=fvb32)

                # transpose fk_w, fv_w first (ekT groups need all of fkT8)
                for w_i, w_ap in enumerate((fkw_d, fvw_d)):
                    for dt in range(DT):
                        st = p0st.tile([P, MD], f32, tag="st")
                        nc.sync.dma_start(out=st,
                                          in_=w_ap[dt * P:(dt + 1) * P, :])
                        st16 = p0st.tile([P, MD], bf16, tag="st16", bufs=2)
                        nc.vector.tensor_copy(out=st16, in_=st)
                        pt = ps_t.tile([P, MT, P], bf16, tag="t")
                        for m in range(MT):
                            nc.tensor.transpose(pt[:, m, :],
                                                st16[:, m * P:(m + 1) * P],
                                                ident16)
                        for m in range(MT):
                            if w_i == 0:
                                nc.vector.tensor_copy(
                                    out=fkT8[:, m // 2, m % 2,
                                             dt * P:(dt + 1) * P],
                                    in_=pt[:, m, :])
                            else:
                                nc.scalar.copy(
                                    out=fvT[m][:, dt * P:(dt + 1) * P],
                                    in_=pt[:, m, :])

                # prefetch chunk 0's kT (independent of the derivation)
                kT8_pre = load_kT(0)

                pv0 = ps_vs.tile([1, 512], f32, tag="vs")
                pv1 = ps_vs.tile([1, 512], f32, tag="vs")
                for jt in range(JT):
                    # mem[jt] -> memT (batched ACT evac; DVE is busy casting)
                    st = p0st.tile([P, MD], f32, tag="st")
                    nc.sync.dma_start(out=st, in_=mem_d[jt * P:(jt + 1) * P, :])
                    st16 = p0st.tile([P, MD], bf16, tag="st16", bufs=2)
                    nc.vector.tensor_copy(out=st16, in_=st)
                    pt = ps_t.tile([P, MT, P], bf16, tag="t")
                    for m in range(MT):
                        nc.tensor.transpose(pt[:, m, :],
                                            st16[:, m * P:(m + 1) * P],
                                            ident16)
                    for m in range(MT):
                        nc.scalar.copy(out=memT[m][:, jt * P:(jt + 1) * P],
                                       in_=pt[:, m, :])
                        nc.vector.tensor_copy(
                            out=memT8[:, m // 2, m % 2, jt * P:(jt + 1) * P],
                            in_=pt[:, m, :])

                    # val8[jt] = relu(sum_m memT[m].T @ fvT[m] + 1 x fv_b)
                    vt16 = p0st.tile([P, D], bf16, tag="vt16", bufs=2)
                    for dh in range(D // 512):
                        ps = ps_s.tile([P, 512], f32, tag="s")
                        for m in range(MT):
                            nc.tensor.matmul(
                                ps,
                                lhsT=memT[m][:, jt * P:(jt + 1) * P],
                                rhs=fvT[m][:, dh * 512:(dh + 1) * 512],
                                start=(m == 0), stop=False)
                        nc.tensor.matmul(
                            ps, lhsT=ones_r16,
                            rhs=fvb16[:, dh * 512:(dh + 1) * 512],
                            start=False, stop=True)
                        nc.scalar.activation(
                            out=vt16[:, dh * 512:(dh + 1) * 512], in_=ps,
                            func=AF.Relu)
                        nc.vector.tensor_scalar_max(
                            val8[:, jt // 2, jt % 2, dh * 512:(dh + 1) * 512],
                            ps, 0.0)
                    nc.tensor.matmul(pv0, lhsT=ones_c16, rhs=vt16[:, 0:512],
                                     start=(jt == 0), stop=(jt == JT - 1))
                    nc.tensor.matmul(pv1, lhsT=ones_c16, rhs=vt16[:, 512:1024],
                                     start=(jt == 0), stop=(jt == JT - 1))

                    if jt % 4 != 3:
                        continue
                    jc = jt // 4
                    # ekT8 groups for this 512-wide j-chunk (fp8 DR)
                    for dt in range(DT):
                        ps = ps_s.tile([P, 512], f32, tag="s")
                        for m2 in range(MT // 2):
                            nc.tensor.matmul(
                                ps,
                                lhsT=fkT8[:, m2, :, dt * P:(dt + 1) * P],
                                rhs=memT8[:, m2, :, jc * 512:(jc + 1) * 512],
                                start=(m2 == 0), stop=(m2 == MT // 2 - 1),
                                perf_mode=DR)
                        nc.scalar.activation(
                            out=ekT8[:, dt // 2, dt % 2,
                                     jc * 512:(jc + 1) * 512],
                            in_=ps, func=AF.Exp, bias=fkbT[:, dt:dt + 1],
                            scale=1.0)
                    # key softmax denominators -> c_col[:, jc*4:(jc+1)*4]
                    pd = ps_d.tile([1, 512], f32, tag="den")
                    for dt in range(DT):
                        nc.tensor.matmul(
                            pd, lhsT=ones_c8,
                            rhs=ekT8[:, dt // 2, dt % 2,
                                     jc * 512:(jc + 1) * 512],
                            start=(dt == 0), stop=(dt == DT - 1))
                    crow = p0st.tile([1, 512], f32, tag="crow", bufs=2)
                    nc.vector.tensor_copy(out=crow, in_=pd)
                    pq = ps_d.tile([P, 4], f32, tag="den")
                    for q in range(4):
                        nc.tensor.transpose(pq[:, q:q + 1],
                                            crow[:, q * P:(q + 1) * P],
                                            ident[0:1, 0:1])
                    nc.vector.reciprocal(
                        out=c_col[:, jc * 4:(jc + 1) * 4], in_=pq)

                # broadcast valsum across partitions (rank-1 fp32 matmul)
                vs_row = p0.tile([1, D], f32, tag="vs_row")
                nc.vector.tensor_copy(out=vs_row[:, 0:512], in_=pv0)
                nc.vector.tensor_copy(out=vs_row[:, 512:1024], in_=pv1)
                for dh in range(D // 512):
                    pb = ps_s.tile([P, 512], f32, tag="s")
                    nc.tensor.matmul(pb, lhsT=ones_r32,
                                     rhs=vs_row[:, dh * 512:(dh + 1) * 512],
                                     start=True, stop=True)
                    nc.vector.tensor_copy(
                        out=vsum_bc[:, dh * 512:(dh + 1) * 512], in_=pb)

            # ---------------- Phase 1: attention over k rows ----------------
            with tc.tile_pool(name="p1", bufs=2) as p1, \
                 tc.tile_pool(name="p1e", bufs=2) as p1e, \
                 tc.tile_pool(name="ps_o", bufs=2, space="PSUM") as ps_o:
                for ci in range(NC_):
                    n0 = ci * NCHUNK
                    kT8 = kT8_pre if ci == 0 else load_kT(ci)

                    # scoresT (DoubleRow) -> exp -> Es = E-1 (fp8)
                    Es8 = p1e.tile([P, JT // 2, 2, NCHUNK], fp8, tag="Es8")
                    for jt in range(JT):
                        ps = ps_s.tile([P, NCHUNK], f32, tag="s")
                        for dc2 in range(DT // 2):
                            nc.tensor.matmul(
                                ps,
                                lhsT=ekT8[:, dc2, :, jt * P:(jt + 1) * P],
                                rhs=kT8[:, dc2, :, :],
                                start=(dc2 == 0), stop=(dc2 == DT // 2 - 1),
                                perf_mode=DR)
                        e16 = p1.tile([P, NCHUNK], bf16, tag="e16", bufs=3)
                        nc.scalar.activation(
                            out=e16, in_=ps, func=AF.Exp,
                            scale=c_col[:, jt:jt + 1])
                        nc.vector.tensor_scalar_add(
                            Es8[:, jt // 2, jt % 2, :], e16, -1.0)

                    # out[n-tile, dd] = (vsum + Es.T @ val) / (J + Es.T @ 1)
                    for ns in range(NS):
                        po0 = ps_o.tile([P, 512], f32, tag="o")
                        po1 = ps_o.tile([P, 512], f32, tag="o")
                        pden = ps_d.tile([P, 1], f32, tag="den")
                        for jc2 in range(JT // 2):
                            lhs = Es8[:, jc2, :, ns * P:(ns + 1) * P]
                            st_, sp_ = (jc2 == 0), (jc2 == JT // 2 - 1)
                            nc.tensor.matmul(po0, lhsT=lhs,
                                             rhs=val8[:, jc2, :, 0:512],
                                             start=st_, stop=sp_, perf_mode=DR)
                            nc.tensor.matmul(po1, lhsT=lhs,
                                             rhs=val8[:, jc2, :, 512:1024],
                                             start=st_, stop=sp_, perf_mode=DR)
                            nc.tensor.matmul(pden, lhsT=lhs,
                                             rhs=ones8[:, :, 0:1],
                                             start=st_, stop=sp_, perf_mode=DR)
                        rv = p1.tile([P, 1], f32, tag="rv")
                        nc.vector.tensor_scalar_add(rv, pden, float(J))
                        nc.vector.reciprocal(out=rv, in_=rv)
                        for dh, po in ((0, po0), (1, po1)):
                            osb = p1.tile([P, 512], f32, tag="osb")
                            nc.vector.tensor_add(
                                osb, po, vsum_bc[:, dh * 512:(dh + 1) * 512])
                            nc.vector.tensor_scalar_mul(osb, osb, rv)
                            nc.sync.dma_start(
                                out=out_d[n0 + ns * P:n0 + (ns + 1) * P,
                                          dh * 512:(dh + 1) * 512],
                                in_=osb)

    nc.compile()
    return nc


def _get_nc():
    if "nc" not in _CACHE:
        _CACHE["nc"] = _build()
    return _CACHE["nc"]


def kernel(**inputs) -> np.ndarray:
    from concourse.bass_utils import run_bass_kernel_spmd

    k = np.ascontiguousarray(np.asarray(inputs["k"], dtype=np.float32))
    mem = np.ascontiguousarray(np.asarray(inputs["mem"], dtype=np.float32))
    fk_w = np.ascontiguousarray(np.asarray(inputs["fk_w"], dtype=np.float32))
    fk_b = np.ascontiguousarray(np.asarray(inputs["fk_b"], dtype=np.float32))
    fv_w = np.ascontiguousarray(np.asarray(inputs["fv_w"], dtype=np.float32))
    fv_b = np.ascontiguousarray(np.asarray(inputs["fv_b"], dtype=np.float32))
    ident = np.eye(P, dtype=np.float32)

    nc = _get_nc()
    in_maps = []
    for c in range(NCORES):
        in_maps.append({
            "k": k[c * S:(c + 1) * S],
            "mem": mem, "fk_w": fk_w, "fk_b": fk_b,
            "fv_w": fv_w, "fv_b": fv_b, "ident": ident,
        })
    res = run_bass_kernel_spmd(nc, in_maps, core_ids=list(range(NCORES)),
                               **_CACHE.get("run_kwargs", {}))
    _CACHE["last_result"] = res
    return np.concatenate([res.results[c]["out"] for c in range(NCORES)],
                          axis=0)
